# revision 1
# baseline (speedup 1.0000x reference)
"""Multi-head attention (RMSNorm-QK + RoPE + softmax + proj) on 8 Trainium2 cores.

Sharding: core c handles batch b = c//4 and heads [3*(c%4), 3*(c%4)+3).
Each core computes qkv for its heads, flash-style attention, and a partial
projection over its heads' channels; the host sums the 4 partials per batch.

Layout tricks (all fp32, matmuls in float32r at 1 cyc/row):
 - q^T/k^T layout [head_dim, tokens]; head-dim rows permuted so the RoPE
   half-swap is an intra-quadrant stream_shuffle.
 - RMS-norm: sum(q^2) via ones-pair matmul; rsqrt = exp(-0.5*ln(x)) so the
   whole kernel uses one ACT table set (natural_log_exp_and_others).
 - softmax without max-subtraction (logits bounded by RMS norm); denominators
   via an appended ones-column in the PV matmul; 1/denom on DVE.
 - qkv/proj biases via K=1 matmul rows.
"""
import sys

for _p in ("/opt/trn_rl_repo", "/opt/trn_rl_repo/concourse"):
    if _p not in sys.path:
        sys.path.insert(0, _p)

import numpy as np
from contextlib import ExitStack

import concourse.bass as bass
import concourse.tile as tile
import concourse.mybir as mybir
from concourse.bass_utils import run_bass_kernel_spmd

F32 = mybir.dt.float32
F32R = mybir.dt.float32r
AF = mybir.ActivationFunctionType

B, N, C = 2, 2048, 768
H, HD = 12, 64
HP = 3            # heads per core
NCORES = 8
CCH = C // 128    # 6 contraction chunks
NT = N // 512     # 4 token tiles of 512
KB = N // 128     # 16 k-blocks of 128
EPS = 1e-6

SWAP_MASK = [(i + 16) % 32 for i in range(32)]
# head-dim permutation: pair-exchange (d <-> d+32) becomes intra-quadrant
PERM = np.concatenate([np.arange(0, 16), np.arange(32, 48),
                       np.arange(16, 32), np.arange(48, 64)])
SIGN = np.where(PERM < 32, -1.0, 1.0).astype(np.float32)

_NC_CACHE = {}


def build_nc(split_waits=True):
    nc = bass.Bass(target_bir_lowering=True)
    xT = nc.declare_dram_parameter("xT", [C, N], F32R, isOutput=False)
    wqk = nc.declare_dram_parameter("wqk", [C, HP * 128], F32R, isOutput=False)
    wv = nc.declare_dram_parameter("wv", [C, 256], F32R, isOutput=False)
    bqk = nc.declare_dram_parameter("bqk", [1, HP * 128], F32R, isOutput=False)
    bv = nc.declare_dram_parameter("bv", [1, 256], F32R, isOutput=False)
    cos2w = nc.declare_dram_parameter("cos2w", [128, N], F32, isOutput=False)
    sinSw = nc.declare_dram_parameter("sinSw", [128, N], F32, isOutput=False)
    sel4 = nc.declare_dram_parameter("sel4", [128, 512], F32R, isOutput=False)
    wp = nc.declare_dram_parameter("wp", [HP * HD, C], F32R, isOutput=False)
    onesd = nc.declare_dram_parameter("onesd", [128, 512], F32R, isOutput=False)
    onespd = nc.declare_dram_parameter("onespd", [128, 2], F32R, isOutput=False)
    vones = nc.declare_dram_parameter("vones", [128, HP * KB], F32R, isOutput=False)
    out = nc.declare_dram_parameter("out", [N, C], F32, isOutput=True)

    with tile.TileContext(nc) as tc, ExitStack() as ctx:
        sb = ctx.enter_context(tc.tile_pool(name="sb", bufs=1))
        tp = ctx.enter_context(tc.tile_pool(name="tp", bufs=2))
        pe = ctx.enter_context(tc.tile_pool(name="pe", bufs=3))   # pexp
        tp1 = ctx.enter_context(tc.tile_pool(name="tp1", bufs=1))
        fps = ctx.enter_context(tc.tile_pool(name="fps", bufs=2, space="PSUM"))
        sA = ctx.enter_context(tc.tile_pool(name="sA", bufs=1, space="PSUM"))
        sB = ctx.enter_context(tc.tile_pool(name="sB", bufs=1, space="PSUM"))
        oA = ctx.enter_context(tc.tile_pool(name="oA", bufs=1, space="PSUM"))
        oB = ctx.enter_context(tc.tile_pool(name="oB", bufs=1, space="PSUM"))


        # ---------- prologue: loads + consts ----------
        wqk_sb, wv_sb, xs = [], [], []
        for c in range(CCH):
            t = sb.tile([128, HP * 128], F32R, tag=f"wqk{c}")
            nc.sync.dma_start(t[:], wqk[c * 128:(c + 1) * 128, :])
            wqk_sb.append(t)
        for c in range(CCH):
            t = sb.tile([128, N], F32R, tag=f"x{c}")
            nc.sync.dma_start(t[:, 0:1024], xT[c * 128:(c + 1) * 128, 0:1024])
            nc.gpsimd.dma_start(t[:, 1024:2048], xT[c * 128:(c + 1) * 128, 1024:2048])
            xs.append(t)
        for c in range(CCH):
            t = sb.tile([128, 256], F32R, tag=f"wv{c}")
            nc.gpsimd.dma_start(t[:], wv[c * 128:(c + 1) * 128, :])
            wv_sb.append(t)
        bqk_sb = sb.tile([1, HP * 128], F32R, tag="bqk")
        nc.sync.dma_start(bqk_sb[:], bqk[:, :])
        bv_sb = sb.tile([1, 256], F32R, tag="bv")
        nc.gpsimd.dma_start(bv_sb[:], bv[:, :])
        cos_sb = sb.tile([128, N], F32, tag="cos")
        nc.gpsimd.dma_start(cos_sb[:], cos2w[:, :])
        sin_sb = sb.tile([128, N], F32, tag="sin")
        nc.gpsimd.dma_start(sin_sb[:], sinSw[:, :])
        sel_sb = sb.tile([128, 512], F32R, tag="sel")
        nc.gpsimd.dma_start(sel_sb[:], sel4[:, :])
        wp0_sb = sb.tile([128, C], F32R, tag="wp0")
        nc.gpsimd.dma_start(wp0_sb[:], wp[0:128, :])
        wp1_sb = sb.tile([64, C], F32R, tag="wp1")
        nc.gpsimd.dma_start(wp1_sb[:], wp[128:192, :])

        ones_row = sb.tile([1, 512], F32R, tag="ones_row")
        nc.gpsimd.dma_start(ones_row[:], onesd[0:1, :])
        onesp = sb.tile([128, 2], F32R, tag="onesp")
        nc.gpsimd.dma_start(onesp[:], onespd[:, :])
        ones64 = sb.tile([1, 64], F32R, tag="ones64")
        nc.gpsimd.dma_start(ones64[:], onesd[0:1, 0:64])
        eps_t = sb.tile([128, 1], F32, tag="eps")
        nc.gpsimd.memset(eps_t[:], EPS)
        v3i = sb.tile([128, HP * KB * 65], F32R, tag="v3i")  # [v_h(kb) | 1] blocks
        nc.gpsimd.dma_start(
            v3i[:].rearrange("p (b n) -> p b n", n=65)[:, :, 64:65],
            vones[:, :, None])

        # qT/kT packed by head pairs so S-matmul operands share a base partition
        q12 = sb.tile([128, N], F32R, tag="q12")   # qT(0) rows 0:64, qT(1) rows 64:128
        k12 = sb.tile([128, N], F32R, tag="k12")
        q3 = sb.tile([64, N], F32R, tag="q3")
        k3 = sb.tile([64, N], F32R, tag="k3")

        def qT(h):
            return (q12[0:64], q12[64:128], q3[:])[h]

        def kT(h):
            return (k12[0:64], k12[64:128], k3[:])[h]

        oall_a = sb.tile([128, N], F32R, tag="oall_a")   # heads 0,1 O^T
        oall_b = sb.tile([64, N], F32R, tag="oall_b")    # head 2 O^T
        t4_all = sb.tile([128, N], F32, tag="t4_all")
        s_sb = sb.tile([128, 512], F32, tag="s_sb")
        nc.gpsimd.memset(s_sb[:], 1.0)
        lnv = sb.tile([128, 512], F32, tag="lnv")
        sv = sb.tile([128, 512], F32R, tag="sv")

        def mm(out_ap, lhsT, rhs, start, stop):
            nc.tensor.matmul(out_ap, lhsT.bitcast(F32R), rhs.bitcast(F32R),
                             start=start, stop=stop, skip_group_check=True)

        # ---------- qkv for head h ----------
        def qkv_passA(h, t):
            ts = slice(t * 512, (t + 1) * 512)
            qk_ps = fps.tile([128, 512], F32, tag="flex")
            for c in range(CCH):
                mm(qk_ps[:], wqk_sb[c][:, h * 128:(h + 1) * 128],
                   xs[c][:, ts], c == 0, False)
            mm(qk_ps[:], bqk_sb[:, h * 128:(h + 1) * 128], ones_row[:],
               False, True)
            t1 = tp1.tile([128, 512], F32, tag="t1")
            nc.vector.tensor_mul(t1[:], qk_ps[:], cos_sb[:, ts])
            t2 = tp.tile([128, 512], F32, tag="t2")
            nc.vector.stream_shuffle(t2[:], qk_ps[:], SWAP_MASK)
            sq = tp.tile([128, 512], F32R, tag="sq")
            nc.vector.tensor_mul(sq[:], t2[:], t2[:])
            t3 = tp1.tile([128, 512], F32, tag="t3")
            nc.vector.tensor_mul(t3[:], t2[:], sin_sb[:, ts])
            mm(qk_ps[0:2, :], onesp[:], sq[:], True, True)
            nc.vector.tensor_copy(s_sb[32 * t:32 * t + 2, :], qk_ps[0:2, :])
            nc.vector.tensor_add(t4_all[:, ts], t1[:], t3[:])

        def qkv_finish(h):
            nc.scalar.activation(lnv[:], s_sb[:], AF.Ln,
                                 bias=eps_t[:], scale=1.0 / HD)
            nc.scalar.activation(sv[:], lnv[:], AF.Exp, bias=0.0, scale=-0.5)
            for t in range(NT):
                ts = slice(t * 512, (t + 1) * 512)
                sqk_ps = fps.tile([128, 512], F32, tag="flex")
                mm(sqk_ps[:], sel_sb[:, t * 128:(t + 1) * 128], sv[:],
                   True, True)
                nc.vector.tensor_mul(qT(h)[:, ts], t4_all[0:64, ts],
                                     sqk_ps[0:64, :])
                nc.vector.tensor_mul(kT(h)[:, ts], t4_all[64:128, ts],
                                     sqk_ps[64:128, :])

        def qkv(h):
            for t in range(NT):
                qkv_passA(h, t)
            qkv_finish(h)

        # ---------- v for all heads ----------
        def vphase_tt(tt):
            v_ps = fps.tile([128, 256], F32, tag="flex")
            for c in range(CCH):
                mm(v_ps[:], xs[c][:, tt * 128:(tt + 1) * 128], wv_sb[c][:],
                   c == 0, False)
            mm(v_ps[:], ones_row[0:1, 0:128], bv_sb[:], False, True)
            # strided copy of 3 head-blocks into v3i (+ ones col at 64)
            dst = v3i[:].rearrange("p (h k n) -> p h k n", h=HP, k=KB)
            nc.vector.tensor_copy(
                dst[:, :, tt, 0:64],
                v_ps[:, 0:192].rearrange("p (h n) -> p h n", h=HP))

        # ---------- attention ----------
        # 16 k-blocks in groups of 2 (one 2-bank PSUM tile per group)
        G2 = [(2 * g, 2 * g + 1) for g in range(8)]

        def epilogue(h, qt, o_ps):
            qs = slice(qt * 512, (qt + 1) * 512)
            ld = tp1.tile([1, 512], F32, tag="ld")
            nc.scalar.activation(ld[:], o_ps[64:65, :], AF.Ln,
                                 bias=0.0, scale=1.0)
            rec = tp1.tile([1, 512], F32R, tag="rec")
            nc.scalar.activation(rec[:], ld[:], AF.Exp, bias=0.0, scale=-1.0)
            rec_ps = fps.tile([64, 512], F32, tag="flex")
            mm(rec_ps[:], ones64[:], rec[:], True, True)
            rec_b = tp1.tile([64, 512], F32, tag="rec_b")
            nc.vector.tensor_copy(rec_b[:], rec_ps[:])
            if h < 2:
                dst = oall_a[h * 64:(h + 1) * 64, qs]
            else:
                dst = oall_b[:, qs]
            nc.vector.tensor_mul(dst, o_ps[0:64, :], rec_b[:])

        def smm(spool, h, kbs, qs):
            s_ps = spool.tile([128, 1024], F32, tag="s")
            for j, kb in enumerate(kbs):
                mm(s_ps[:, j * 512:(j + 1) * 512],
                   kT(h)[:, kb * 128:(kb + 1) * 128], qT(h)[:, qs], True, True)
            return s_ps

        def pexp_of(s_ps):
            px = pe.tile([128, 1024], F32R, tag="pexp")
            nc.scalar.activation(px[:], s_ps[:], AF.Exp, bias=0.0, scale=0.125)
            return px

        def omm(o_ps, h, kbs, px):
            for j, kb in enumerate(kbs):
                mm(o_ps[:], v3i[:, (h * KB + kb) * 65:(h * KB + kb) * 65 + 65],
                   px[:, j * 512:(j + 1) * 512], kb == 0, kb == KB - 1)

        # ---------- partial projection (token tiles of one q-tile) ----------
        def proj_qt(qt):
            for tt in range(4 * qt, 4 * qt + 4):
                po = tp.tile([128, C], F32, tag="po")
                for half in range(2):
                    cs = slice(half * 384, (half + 1) * 384)
                    p_ps = fps.tile([128, 512], F32, tag="flex")
                    mm(p_ps[:, 0:384], oall_a[:, tt * 128:(tt + 1) * 128],
                       wp0_sb[:, cs], True, False)
                    mm(p_ps[:, 0:384], oall_b[:, tt * 128:(tt + 1) * 128],
                       wp1_sb[:, cs], False, True)
                    nc.vector.tensor_copy(po[:, cs], p_ps[:, 0:384])
                nc.sync.dma_start(out[tt * 128:(tt + 1) * 128, :], po[:])


        def attn_single(h, extra=None):
            for qt in range(NT):
                qs = slice(qt * 512, (qt + 1) * 512)
                o_ps = (oA if qt % 2 == 0 else oB).tile([65, 512], F32, tag="o")
                for g, kbs in enumerate(G2):
                    s_ps = smm(sA if g % 2 == 0 else sB, h, kbs, qs)
                    px = pexp_of(s_ps)
                    omm(o_ps, h, kbs, px)
                epilogue(h, qt, o_ps)
                if extra is not None:
                    extra(qt)

        def attn_pair(h0, h1):
            # h0/h1 S-matmuls sit in different PE row-groups (base partition
            # 0 vs 64) and different PSUM banks -> they run concurrently.
            for qt in range(NT):
                qs = slice(qt * 512, (qt + 1) * 512)
                o0 = oA.tile([65, 512], F32, tag="o")
                o1 = oB.tile([65, 512], F32, tag="o")
                for kbs in G2:
                    s0 = smm(sA, h0, kbs, qs)
                    s1 = smm(sB, h1, kbs, qs)
                    px0 = pexp_of(s0)
                    omm(o0, h0, kbs, px0)
                    px1 = pexp_of(s1)
                    omm(o1, h1, kbs, px1)
                epilogue(h0, qt, o0)
                epilogue(h1, qt, o1)
                proj_qt(qt)

        def prep_next(qt):
            if qt == 0:
                qkv_passA(1, 0)
            elif qt == 1:
                qkv_passA(1, 1)
                qkv_passA(1, 2)
                qkv_passA(1, 3)
            elif qt == 2:
                qkv_finish(1)
                qkv_passA(2, 0)
                qkv_passA(2, 1)
            else:
                qkv_passA(2, 2)
                qkv_passA(2, 3)
                qkv_finish(2)

        qkv(0)
        for tt in range(KB):
            vphase_tt(tt)
        attn_single(0, extra=prep_next)
        attn_pair(1, 2)

    if split_waits:
        _split_waits(nc)
    return nc


def _split_waits(nc):
    """This walrus build lowers at most one sync-wait per instruction (the
    matmul LDW struct rejects 2+). Move excess waits onto NoOps inserted
    just before, on the same engine queue — queues are in-order, so the
    constraint is preserved exactly."""
    k = 0
    for fn in nc.m.functions:
        for bb in fn.blocks:
            il = bb.instructions
            idx = 0
            while idx < len(il):
                inst = il[idx]
                si = inst.sync_info
                eng = getattr(inst, "engine", None)
                if (si is not None and len(si.on_wait) > 1
                        and eng is not None
                        and str(eng) != "EngineType.Unassigned"):
                    waits = list(si.on_wait)
                    inst.sync_info = mybir.SyncInfo(
                        on_wait=[waits[-1]], on_update=list(si.on_update))
                    for w in waits[:-1]:
                        nop = mybir.InstNoOp(
                            name=f"I-waitnop-{k}", engine=eng, ins=[], outs=[],
                            sync_info=mybir.SyncInfo(on_wait=[w], on_update=[]))
                        k += 1
                        il.insert(idx, nop)
                        idx += 1
                idx += 1


def _prep_core_inputs(core, x, rope_cos, rope_sin, qkv_kernel, qkv_bias,
                      proj_kernel, proj_bias, q_norm_w, k_norm_w):
    b = core // 4
    heads = [3 * (core % 4) + i for i in range(HP)]

    wq = qkv_kernel.reshape(C, 3, H, HD)
    bq = qkv_bias.reshape(3, H, HD)

    xT = np.ascontiguousarray(x[b].T, dtype=np.float32)

    wqk = np.empty((C, HP * 128), np.float32)
    bqk = np.empty((1, HP * 128), np.float32)
    for i, h in enumerate(heads):
        wqk[:, i * 128:i * 128 + 64] = wq[:, 0, h, PERM]
        wqk[:, i * 128 + 64:(i + 1) * 128] = wq[:, 1, h, PERM]
        bqk[0, i * 128:i * 128 + 64] = bq[0, h, PERM]
        bqk[0, i * 128 + 64:(i + 1) * 128] = bq[1, h, PERM]

    wv = np.zeros((C, 256), np.float32)
    bv = np.zeros((1, 256), np.float32)
    for i, h in enumerate(heads):
        wv[:, i * 64:(i + 1) * 64] = wq[:, 2, h, :]
        bv[0, i * 64:(i + 1) * 64] = bq[2, h, :]

    cosT = rope_cos.T  # (HD, N)
    sinT = rope_sin.T
    cos2w = np.empty((128, N), np.float32)
    sinSw = np.empty((128, N), np.float32)
    cos2w[0:64] = cosT[PERM] * q_norm_w[PERM][:, None]
    cos2w[64:128] = cosT[PERM] * k_norm_w[PERM][:, None]
    sinSw[0:64] = SIGN[:, None] * sinT[PERM] * q_norm_w[PERM][:, None]
    sinSw[64:128] = SIGN[:, None] * sinT[PERM] * k_norm_w[PERM][:, None]

    onesd = np.ones((128, 512), np.float32)
    onespd = np.zeros((128, 2), np.float32)
    onespd[0:64, 0] = 1.0    # col0: ones on q rows
    onespd[64:128, 1] = 1.0  # col1: ones on k rows
    vones = np.ones((128, HP * KB), np.float32)

    sel4 = np.zeros((128, 512), np.float32)
    for t in range(NT):
        sel4[32 * t, t * 128:t * 128 + 64] = 1.0
        sel4[32 * t + 1, t * 128 + 64:(t + 1) * 128] = 1.0

    rows = np.concatenate([np.arange(h * HD, (h + 1) * HD) for h in heads])
    wp = np.ascontiguousarray(proj_kernel[rows, :], dtype=np.float32)

    return {"xT": xT, "wqk": wqk, "wv": wv, "bqk": bqk, "bv": bv,
            "cos2w": cos2w, "sinSw": sinSw, "sel4": sel4,
            "wp": wp, "onesd": onesd, "onespd": onespd, "vones": vones}


def kernel(x, rope_cos, rope_sin, qkv_kernel, qkv_bias, proj_kernel,
           proj_bias, q_norm_w, k_norm_w, _trace=False):
    args = [np.asarray(a, dtype=np.float32) for a in
            (x, rope_cos, rope_sin, qkv_kernel, qkv_bias, proj_kernel,
             proj_bias, q_norm_w, k_norm_w)]
    in_maps = [_prep_core_inputs(c, *args) for c in range(NCORES)]

    if "nc" not in _NC_CACHE:
        _NC_CACHE["nc"] = build_nc()
    nc = _NC_CACHE["nc"]

    res = run_bass_kernel_spmd(nc, in_maps, core_ids=list(range(NCORES)),
                               trace=_trace)
    parts = [res.results[c]["out"] for c in range(NCORES)]
    out = np.empty((B, N, C), np.float32)
    pb = np.asarray(proj_bias, dtype=np.float32)
    for b in range(B):
        out[b] = parts[4 * b] + parts[4 * b + 1] + parts[4 * b + 2] + parts[4 * b + 3] + pb
    if _trace:
        kernel.last_results = res
    return out



# revision 2
# speedup vs baseline: 1.5087x; 1.5087x over previous
"""Multi-head attention (RMSNorm-QK + RoPE + softmax + proj) on 8 Trainium2 cores.

v2 design (cost-model-driven rewrite of the baseline):
 - bf16 operands everywhere (matmuls cost 1 cyc/row like fp32r, but DVE gets
   2x modes and DMA halves); fp32 PSUM accumulation throughout.
 - Transposed PV: O tiles are [128 q, 65] (64 dims + ones col for the softmax
   denominator), using all 128 output partitions -> PV drops from 32768 to
   16640 cyc/head, the denominator becomes a per-partition column (one DVE
   tensor_scalar divide), and the old broadcast-reciprocal matmuls vanish.
 - O^T for the projection comes from PE transposes (128 bf16 rows each).
 - RMS rsqrt on DVE ((x/64)^-0.5 via tensor_scalar pow), qkv bias added in the
   DVE pipeline (per-partition scalar), v bias folded into the host-side proj
   bias (softmax rows sum to 1), so ACT runs the softmax exp ONLY.
 - RoPE elementwise work split DVE/Pool; emission order software-pipelines
   S(k+1) ahead of exp(k), stages a phase's px tiles in SBUF so each O
   qb-region accumulates contiguously (PSUM start bit stays per-element
   correct on HW), defers phase closes ~1.25 phases so early PE work (qkv+v)
   overlaps the ACT-bound exp stream, and pumps qkv/v/proj filler chunks into
   the PE gaps.

Sharding: core c handles batch c//4 and heads [3*(c%4), 3*(c%4)+3).
Each core writes a bf16 [N, C] partial; the host sums 4 partials per batch
and adds proj_bias + qkv_bias[v-part] @ proj_kernel.
"""
import sys

for _p in ("/opt/trn_rl_repo", "/opt/trn_rl_repo/concourse"):
    if _p not in sys.path:
        sys.path.insert(0, _p)

from collections import deque
from contextlib import ExitStack

import ml_dtypes
import numpy as np

import concourse.bass as bass
import concourse.mybir as mybir
import concourse.tile as tile
from concourse.bass_utils import run_bass_kernel_spmd

F32 = mybir.dt.float32
BF16 = mybir.dt.bfloat16
AF = mybir.ActivationFunctionType
ALU = mybir.AluOpType
BF = ml_dtypes.bfloat16

B, N, C = 2, 2048, 768
H, HD = 12, 64
HP = 3            # heads per core
NCORES = 8
CCH = 6           # contraction chunks of 128
NT = 4            # token tiles of 512
KB = 16           # k blocks of 128
NG = 8            # 2-kb groups per (head, qtile) phase

SWAP_MASK = [(i + 16) % 32 for i in range(32)]
PERM = np.concatenate([np.arange(0, 16), np.arange(32, 48),
                       np.arange(16, 32), np.arange(48, 64)])
SIGN = np.where(PERM < 32, -1.0, 1.0).astype(np.float32)
# rope partner of PERM-position p (SWAP_MASK's intra-32 half swap)
SWAPIDX = np.array([(p // 32) * 32 + (p + 16) % 32 for p in range(64)])

_NC_CACHE = {}


def build_nc(split_waits=True):
    nc = bass.Bass(target_bir_lowering=True)
    xT = nc.declare_dram_parameter("xT", [C, N], BF16, isOutput=False)
    wqk = nc.declare_dram_parameter("wqk", [C, HP * 128], BF16, isOutput=False)
    cosw = nc.declare_dram_parameter("cosw", [128, N], BF16, isOutput=False)
    sinw = nc.declare_dram_parameter("sinw", [128, N], BF16, isOutput=False)
    wvp = nc.declare_dram_parameter("wvp", [128, CCH * HP * 64], BF16,
                                    isOutput=False)
    wp01 = nc.declare_dram_parameter("wp01", [128, C], BF16, isOutput=False)
    wp2 = nc.declare_dram_parameter("wp2", [64, C], BF16, isOutput=False)
    # consts: [onesp(2) | sel4(512) | ident(128)]
    consts = nc.declare_dram_parameter("consts", [128, 642], BF16,
                                       isOutput=False)
    bqk = nc.declare_dram_parameter("bqk", [128, HP], F32, isOutput=False)
    out = nc.declare_dram_parameter("out", [N, C], BF16, isOutput=True)

    with tile.TileContext(nc) as tc, ExitStack() as ctx:
        sb = ctx.enter_context(tc.tile_pool(name="sb", bufs=1))
        pipe = ctx.enter_context(tc.tile_pool(name="pipe", bufs=2))
        pxp = ctx.enter_context(tc.tile_pool(name="pxp", bufs=28))
        otp = ctx.enter_context(tc.tile_pool(name="otp", bufs=4))
        pop = ctx.enter_context(tc.tile_pool(name="pop", bufs=2))
        # PSUM: 4 + 2 + 1 + 1 = 8 banks
        sp = ctx.enter_context(tc.tile_pool(name="sp", bufs=2, space="PSUM"))
        qp = ctx.enter_context(tc.tile_pool(name="qp", bufs=2, space="PSUM"))
        op = ctx.enter_context(tc.tile_pool(name="op", bufs=1, space="PSUM"))
        mp = ctx.enter_context(tc.tile_pool(name="mp", bufs=1, space="PSUM"))

        # ---------- static SBUF tiles ----------
        xs = sb.tile([128, CCH, N], BF16, tag="xs")
        wqk_sb = sb.tile([128, CCH, HP * 128], BF16, tag="wqk")
        wv_sb = sb.tile([128, CCH, HP * 64], BF16, tag="wv")
        cos_sb = sb.tile([128, N], BF16, tag="cos")
        sin_sb = sb.tile([128, N], BF16, tag="sin")
        cn = sb.tile([128, 642], BF16, tag="cn")
        onesp_sb = cn[:, 0:2]
        sel_sb = cn[:, 2:514]
        ident_sb = cn[:, 514:642]
        bqk_sb = sb.tile([128, HP], F32, tag="bqk")
        wp01_sb = sb.tile([128, C], BF16, tag="wp01")
        wp2_sb = sb.tile([64, C], BF16, tag="wp2")

        q12 = sb.tile([128, N], BF16, tag="q12")
        k12 = sb.tile([128, N], BF16, tag="k12")
        q3 = sb.tile([64, N], BF16, tag="q3")
        k3 = sb.tile([64, N], BF16, tag="k3")
        t4_all = sb.tile([128, N], BF16, tag="t4_all")
        s_sb = sb.tile([128, 512], F32, tag="s_sb")
        sv = sb.tile([128, 512], BF16, tag="sv")
        v3 = sb.tile([128, KB, HP, 65], BF16, tag="v3")
        ones48 = sb.tile([128, KB * HP], BF16, tag="ones48")
        o2 = sb.tile([128, NT, 4, 128], BF16, tag="o2")
        o1 = sb.tile([128, NT, 4, 64], BF16, tag="o1")

        def qT(h):
            return (q12[0:64], q12[64:128], q3[:])[h]

        def kT(h):
            return (k12[0:64], k12[64:128], k3[:])[h]

        # ---------- prologue DMAs (ordered for earliest qkv start) ----------
        xT_r = xT[:].rearrange("(c p) n -> p c n", p=128)
        wqk_r = wqk[:].rearrange("(c p) m -> p c m", p=128)
        d = nc.sync.dma_start
        d(cn[:], consts[:, :])
        d(bqk_sb[:], bqk[:, :])
        d(wqk_sb[:, 0:2, :], wqk_r[:, 0:2, :])
        d(xs[:, 0:3, 0:512], xT_r[:, 0:3, 0:512])      # tile-0 tokens
        d(wqk_sb[:, 2:6, :], wqk_r[:, 2:6, :])
        d(xs[:, 3:6, 0:512], xT_r[:, 3:6, 0:512])
        d(cos_sb[:, 0:1024], cosw[:, 0:1024])
        d(sin_sb[:, 0:1024], sinw[:, 0:1024])
        d(xs[:, :, 512:1024], xT_r[:, :, 512:1024])
        d(xs[:, :, 1024:1536], xT_r[:, :, 1024:1536])
        d(wv_sb[:].rearrange("p c m -> p (c m)"), wvp[:, :])
        d(xs[:, :, 1536:2048], xT_r[:, :, 1536:2048])
        d(cos_sb[:, 1024:2048], cosw[:, 1024:2048])
        d(sin_sb[:, 1024:2048], sinw[:, 1024:2048])
        d(wp01_sb[:], wp01[:, :])
        d(wp2_sb[:], wp2[:, :])

        nc.vector.memset(sv[:], 1.0)   # rows never written stay 1 (sel zeros them)
        wz = mp.tile([128, 128], F32, tag="m", name="wz")
        for _ in range(14):
            nc.tensor.matmul(wz[:], sv[:, 0:128], sv[:, 0:128], start=True,
                             stop=True, skip_group_check=True)
        nc.vector.memset(s_sb[:], 1.0)
        nc.vector.memset(ones48[:], 1.0)
        nc.vector.tensor_copy(
            v3[:].rearrange("p a b n -> p (a b) n", n=65)[:, :, 64], ones48[:])

        def mm(out_ap, lhsT, rhs, start, stop):
            nc.tensor.matmul(out_ap, lhsT, rhs, start=start, stop=stop,
                             skip_group_check=True)

        # ---------- qkv-head generator ----------
        # Per-tile chunks: mms -> RoPE pipe -> finA (sumsq+rsqrt) ->
        # finB (broadcast+scale). Emission defers fins so PE stays dense;
        # the qp ring (2) tolerates exactly one deferred finA.
        def qkv_gen(h):
            hs = slice(h * 128, (h + 1) * 128)
            qk = [None] * NT

            def mms(t):
                ts = slice(t * 512, (t + 1) * 512)
                qk[t] = qp.tile([128, 512], F32, tag="q", name=f"qk{t}")
                for c in range(CCH):
                    mm(qk[t][:], wqk_sb[:, c, hs], xs[:, c, ts], c == 0,
                       c == CCH - 1)

            def rope(t):
                ts = slice(t * 512, (t + 1) * 512)
                qkb = pipe.tile([128, 512], BF16, tag="qkb")
                nc.vector.tensor_scalar(qkb[:], qk[t][:], bqk_sb[:, h:h + 1],
                                        None, ALU.add)
                t1 = pipe.tile([128, 512], BF16, tag="t1")
                nc.gpsimd.tensor_mul(t1[:], qkb[:], cos_sb[:, ts])
                t2 = pipe.tile([128, 512], BF16, tag="t2")
                nc.vector.stream_shuffle(t2[:], qkb[:], SWAP_MASK)
                t3 = pipe.tile([128, 512], BF16, tag="t3")
                nc.vector.tensor_mul(t3[:], t2[:], sin_sb[:, ts])
                nc.vector.tensor_add(t4_all[:, ts], t1[:], t3[:])
                sq = pipe.tile([128, 512], BF16, tag="sq")
                if h == 0:
                    nc.vector.tensor_mul(sq[:], qkb[:], qkb[:])
                else:
                    nc.gpsimd.tensor_mul(sq[:], qkb[:], qkb[:])
                return sq

            def finA(t, sq):
                rows = slice(32 * t, 32 * t + 2)
                mm(qk[t][0:2, :], onesp_sb[:], sq[:], True, True)
                if h == 0:
                    # rsqrt = exp(-0.5 ln(ms)); same ACT table as softmax exp
                    lv = pipe.tile([2, 512], F32, tag="lv", name="lv")
                    nc.scalar.activation(lv[:], qk[t][0:2, :], AF.Ln,
                                         bias=0.0, scale=1.0 / HD)
                    nc.scalar.activation(sv[rows, :], lv[:], AF.Exp,
                                         bias=0.0, scale=-0.5)
                else:
                    nc.vector.tensor_copy(s_sb[rows, :], qk[t][0:2, :])

            def lnexp():
                lva = pipe.tile([128, 512], F32, tag="lva", name="lva")
                nc.scalar.activation(lva[:], s_sb[:], AF.Ln,
                                     bias=0.0, scale=1.0 / HD)
                nc.scalar.activation(sv[:], lva[:], AF.Exp, bias=0.0,
                                     scale=-0.5)

            def finB(t):
                ts = slice(t * 512, (t + 1) * 512)
                sqk_ps = qp.tile([128, 512], F32, tag="q")
                mm(sqk_ps[:], sel_sb[:, t * 128:(t + 1) * 128], sv[:],
                   True, True)
                sqk_sb = pipe.tile([128, 512], BF16, tag="sqk")
                nc.vector.tensor_copy(sqk_sb[:], sqk_ps[:])
                nc.vector.tensor_mul(qT(h)[:, ts], t4_all[0:64, ts],
                                     sqk_sb[0:64, :])
                nc.vector.tensor_mul(kT(h)[:, ts], t4_all[64:128, ts],
                                     sqk_sb[64:128, :])

            sqs = [None] * NT

            def do_mms(t):
                mms(t)
                sqs[t] = rope(t)

            do_mms(0)
            yield 4500
            do_mms(1)
            yield 4500
            finA(0, sqs[0])
            yield 700
            do_mms(2)
            yield 4500
            finA(1, sqs[1])
            if h == 0:
                finB(0)
            yield 1600
            finA(2, sqs[2])
            yield 700
            do_mms(3)
            yield 4500
            if h == 0:
                finB(1)
            yield 900
            finA(3, sqs[3])
            yield 700
            if h != 0:
                lnexp()
                yield 700
                finB(0)
                yield 900
                finB(1)
                yield 900
            finB(2)
            yield 900
            finB(3)
            yield 900

        # ---------- v generator ----------
        def v_gen():
            for tt in range(KB):
                v_ps = qp.tile([128, HP * 64], F32, tag="q")
                for c in range(CCH):
                    mm(v_ps[:], xs[:, c, tt * 128:(tt + 1) * 128],
                       wv_sb[:, c, :], c == 0, c == CCH - 1)
                nc.vector.tensor_copy(
                    v3[:, tt, :, 0:64],
                    v_ps[:, :].rearrange("p (h n) -> p h n", h=HP))
                yield 1500

        # ---------- proj of one (qtile, qblock) ----------
        mtr = [None]

        def proj_qb(qt, qb):
            if mtr[0] is None:
                mtr[0] = mp.tile([128, 4, 128], F32, tag="m", name="mtr")
            m = mtr[0]
            tr01 = m[:, qb, 0:64].bitcast(BF16)
            tr2 = m[0:64, qb, 64:128].bitcast(BF16)
            nc.tensor.transpose(tr01, o2[:, qt, qb, :], ident_sb[:])
            nc.tensor.transpose(tr2, o1[:, qt, qb, :], ident_sb[:])
            on_act = qt == 3   # ACT is idle once the last exps drain
            ot01 = otp.tile([128, 128], BF16, tag="ot01")
            ot2 = otp.tile([64, 128], BF16, tag="ot2")
            if on_act:
                nc.scalar.activation(ot01[:], tr01, AF.Copy, bias=0.0,
                                     scale=1.0)
                nc.vector.tensor_copy(ot2[:], tr2)
            else:
                nc.vector.tensor_copy(ot01[:], tr01)
                nc.vector.tensor_copy(ot2[:], tr2)
            po = pop.tile([128, C], BF16, tag="po")
            for half in range(2):
                cs = slice(half * 384, (half + 1) * 384)
                p_ps = qp.tile([128, 384], F32, tag="q")
                mm(p_ps[:], ot01[:], wp01_sb[:, cs], True, False)
                mm(p_ps[:], ot2[:], wp2_sb[:, cs], False, True)
                if on_act and half == 1:
                    nc.scalar.activation(po[:, cs], p_ps[:], AF.Copy,
                                         bias=0.0, scale=1.0)
                else:
                    nc.vector.tensor_copy(po[:, cs], p_ps[:])
            tb = qt * 4 + qb
            nc.sync.dma_start(out[tb * 128:(tb + 1) * 128, :], po[:])

        # ---------- filler pump ----------
        fillers = deque()
        debt = [0.0]

        def pump(budget):
            budget += debt[0]
            while budget > 0 and fillers:
                try:
                    budget -= next(fillers[0])
                except StopIteration:
                    fillers.popleft()
            debt[0] = min(budget, 3000.0)

        def ensure_done(gen):
            """Pump until `gen` has fully emitted (emission-order guard for
            cross-generator data deps)."""
            while gen in fillers:
                pump(100000)

        # ---------- attention stream ----------
        phases = [(h, qt) for h in range(HP) for qt in range(NT)]
        px_tiles = {}
        emitted = set()

        def emit_group(p, g):
            if (p, g) in emitted:
                return
            emitted.add((p, g))
            h, qt = phases[p]
            qs = slice(qt * 512, (qt + 1) * 512)
            s_ps = sp.tile([128, 1024], F32, tag="s")
            for j in range(2):
                kb = 2 * g + j
                mm(s_ps[:, j * 512:(j + 1) * 512],
                   kT(h)[:, kb * 128:(kb + 1) * 128], qT(h)[:, qs],
                   True, True)
            px = pxp.tile([128, 1024], BF16, tag="px")
            nc.scalar.activation(px[:], s_ps[:], AF.Exp, bias=0.0, scale=0.125)
            px_tiles[(p, g)] = px

        def close_gen(p):
            if p == 0:
                ensure_done(vg)   # PV reads v3; emission-order guard
            h, qt = phases[p]
            o_ps = op.tile([128, 4, 65], F32, tag="o")
            for qb in range(4):
                for g in range(NG):
                    px = px_tiles[(p, g)]
                    for j in range(2):
                        kb = 2 * g + j
                        mm(o_ps[:, qb, :],
                           px[:, j * 512 + qb * 128:j * 512 + (qb + 1) * 128],
                           v3[:, kb, h, :],
                           qb == 0 and kb == 0, kb == KB - 1)
                if qb == 1 or qb == 3:
                    yield
            # normalize by the ones-column denominators (batched reciprocal,
            # then per-qb per-partition multiply); epilogues after ALL PV so
            # coarse WAR tracking can't serialize the qb bundles
            rec4 = pipe.tile([128, 4], F32, tag="rec4", name="rec4")
            nc.vector.reciprocal(rec4[:], o_ps[:, :, 64])
            for qb in range(4):
                dst = (o2[:, qt, qb, h * 64:(h + 1) * 64] if h < 2
                       else o1[:, qt, qb, :])
                nc.vector.tensor_scalar(dst, o_ps[:, qb, 0:64],
                                        rec4[:, qb:qb + 1], None, ALU.mult)
                if h == 2:
                    proj_qb(qt, qb)
                yield
            for g in range(NG):
                del px_tiles[(p, g)]

        def drain_close(cg):
            for _ in cg:
                pass

        # ---------- main schedule ----------
        # Phase 0 runs with qkv(h0) inlined per tile: tile t unlocks S groups
        # 2t, 2t+1 (k-tiles) while qT(qt0) comes entirely from tile 0.
        qg0 = qkv_gen(0)
        vg = v_gen()
        g1, g2 = qkv_gen(1), qkv_gen(2)
        fillers.append(vg)
        fillers.append(g1)
        need_gen = {1: g1, 2: g2}

        def drain_n(gen, n_chunks):
            for _ in range(n_chunks):
                next(gen)

        # phase 0: dense qkv-h0 mm front with S groups as tiles finish
        drain_n(qg0, 5)              # thru finB(0): groups 0-1 ready
        for g in range(NG):
            if g == 2:
                drain_n(qg0, 3)      # finA2, mms3, finB1
            elif g == 4:
                drain_n(qg0, 2)      # finA3, finB2
            elif g == 6:
                drain_n(qg0, 1)      # finB3
            emit_group(0, g)
            pump(900)

        # Steady state: early closes deferred ~2 phases (px ring holds ~3
        # phases) so v/qkv fillers use the early PE slack; late closes pulled
        # in so proj work overlaps the remaining exp stream.
        NP = len(phases)
        close_at = {}
        for p in range(NP - 1):
            # early closes deferred 3 phases (shifts PV out of the PE-heavy
            # qkv/v window); h2 closes pulled in so proj overlaps exps
            lag = (p + 3, 1) if p < 8 else (p + 1, 0)
            close_at.setdefault(lag, []).append(p)

        def after_close(cp):
            if phases[cp] == (0, 3):
                fillers.append(g2)

        active_closes = []
        for p in range(1, NP):
            nh = phases[p][0]
            if nh != phases[p - 1][0]:
                ensure_done(need_gen[nh])
            for g in range(NG):
                emit_group(p, g)
                pump(1800 if p <= 4 else 1300)
                for cp in close_at.get((p, g), []):
                    active_closes.append((cp, close_gen(cp)))
                if active_closes:
                    cp, cg = active_closes[0]
                    try:
                        next(cg)
                    except StopIteration:
                        active_closes.pop(0)
                        after_close(cp)
        for cp, cg in active_closes:
            drain_close(cg)
            after_close(cp)
        drain_close(close_gen(NP - 1))
        while fillers:
            pump(100000)

    if split_waits:
        _split_waits(nc)
    return nc


def _split_waits(nc):
    """Walrus lowers at most one sync-wait per instruction; move excess waits
    onto NoOps inserted just before, on the same engine queue."""
    k = 0
    for fn in nc.m.functions:
        for bb in fn.blocks:
            il = bb.instructions
            idx = 0
            while idx < len(il):
                inst = il[idx]
                si = inst.sync_info
                eng = getattr(inst, "engine", None)
                if (si is not None and len(si.on_wait) > 1
                        and eng is not None
                        and str(eng) != "EngineType.Unassigned"):
                    waits = list(si.on_wait)
                    inst.sync_info = mybir.SyncInfo(
                        on_wait=[waits[-1]], on_update=list(si.on_update))
                    for w in waits[:-1]:
                        nop = mybir.InstNoOp(
                            name=f"I-waitnop-{k}", engine=eng, ins=[], outs=[],
                            sync_info=mybir.SyncInfo(on_wait=[w], on_update=[]))
                        k += 1
                        il.insert(idx, nop)
                        idx += 1
                idx += 1


def _prep_core_inputs(core, x, rope_cos, rope_sin, qkv_kernel, qkv_bias,
                      proj_kernel, proj_bias, q_norm_w, k_norm_w):
    b = core // 4
    heads = [3 * (core % 4) + i for i in range(HP)]

    wq = qkv_kernel.reshape(C, 3, H, HD)
    bq = qkv_bias.reshape(3, H, HD)

    xTa = np.ascontiguousarray(x[b].T).astype(BF)

    wqk = np.empty((C, HP * 128), np.float32)
    bqk = np.zeros((128, HP), np.float32)
    for i, h in enumerate(heads):
        wqk[:, i * 128:i * 128 + 64] = wq[:, 0, h, PERM]
        wqk[:, i * 128 + 64:(i + 1) * 128] = wq[:, 1, h, PERM]
        bqk[0:64, i] = bq[0, h, PERM]
        bqk[64:128, i] = bq[1, h, PERM]

    wv = np.zeros((C, HP * 64), np.float32)
    for i, h in enumerate(heads):
        wv[:, i * 64:(i + 1) * 64] = wq[:, 2, h, :]
    # packed [p, (c m)] so the SBUF copy is one dense DMA
    wvp = wv.reshape(CCH, 128, HP * 64).transpose(1, 0, 2).reshape(128, -1)

    cosT = rope_cos.T  # (HD, N)
    sinT = rope_sin.T
    cosw = np.empty((128, N), np.float32)
    sinw = np.empty((128, N), np.float32)
    cosw[0:64] = cosT[PERM] * q_norm_w[PERM][:, None]
    cosw[64:128] = cosT[PERM] * k_norm_w[PERM][:, None]
    # sin multiplies the SHUFFLED (partner) value -> partner's norm weight
    qn_p = q_norm_w[PERM][SWAPIDX]
    kn_p = k_norm_w[PERM][SWAPIDX]
    sinw[0:64] = SIGN[:, None] * sinT[PERM] * qn_p[:, None]
    sinw[64:128] = SIGN[:, None] * sinT[PERM] * kn_p[:, None]

    onesp = np.zeros((128, 2), np.float32)
    onesp[0:64, 0] = 1.0
    onesp[64:128, 1] = 1.0

    sel4 = np.zeros((128, 512), np.float32)
    for t in range(NT):
        sel4[32 * t, t * 128:t * 128 + 64] = 1.0
        sel4[32 * t + 1, t * 128 + 64:(t + 1) * 128] = 1.0

    rows01 = np.concatenate([np.arange(h * HD, (h + 1) * HD)
                             for h in heads[0:2]])
    rows2 = np.arange(heads[2] * HD, (heads[2] + 1) * HD)
    wp01 = proj_kernel[rows01, :]
    wp2 = proj_kernel[rows2, :]

    consts = np.zeros((128, 642), np.float32)
    consts[:, 0:2] = onesp
    consts[:, 2:514] = sel4
    consts[:, 514:642] = np.eye(128, dtype=np.float32)
    return {"xT": xTa, "wqk": wqk.astype(BF), "bqk": bqk,
            "cosw": cosw.astype(BF), "sinw": sinw.astype(BF),
            "wvp": np.ascontiguousarray(wvp).astype(BF),
            "wp01": np.ascontiguousarray(wp01).astype(BF),
            "wp2": np.ascontiguousarray(wp2).astype(BF),
            "consts": consts.astype(BF)}


def kernel(x, rope_cos, rope_sin, qkv_kernel, qkv_bias, proj_kernel,
           proj_bias, q_norm_w, k_norm_w, _trace=False):
    args = [np.asarray(a, dtype=np.float32) for a in
            (x, rope_cos, rope_sin, qkv_kernel, qkv_bias, proj_kernel,
             proj_bias, q_norm_w, k_norm_w)]
    in_maps = [_prep_core_inputs(c, *args) for c in range(NCORES)]

    if "nc" not in _NC_CACHE:
        _NC_CACHE["nc"] = build_nc()
    nc = _NC_CACHE["nc"]

    res = run_bass_kernel_spmd(nc, in_maps, core_ids=list(range(NCORES)),
                               trace=_trace)
    parts = [np.asarray(res.results[c]["out"]).astype(np.float32)
             for c in range(NCORES)]
    # v-bias contributes exactly bv @ proj_kernel (softmax rows sum to 1)
    pb = (np.asarray(proj_bias, dtype=np.float32)
          + np.asarray(qkv_bias, dtype=np.float32)[2 * C:]
          @ np.asarray(proj_kernel, dtype=np.float32))
    out = np.empty((B, N, C), np.float32)
    for b in range(B):
        out[b] = (parts[4 * b] + parts[4 * b + 1] + parts[4 * b + 2]
                  + parts[4 * b + 3] + pb)
    if _trace:
        kernel.last_results = res
    return out


# revision 3
# speedup vs baseline: 1.5290x; 1.0135x over previous
"""Multi-head attention (RMSNorm-QK + RoPE + softmax + proj) on 8 Trainium2 cores.

v2 design (cost-model-driven rewrite of the baseline):
 - bf16 operands everywhere (matmuls cost 1 cyc/row like fp32r, but DVE gets
   2x modes and DMA halves); fp32 PSUM accumulation throughout.
 - Transposed PV: O tiles are [128 q, 65] (64 dims + ones col for the softmax
   denominator), using all 128 output partitions -> PV drops from 32768 to
   16640 cyc/head, the denominator becomes a per-partition column (one DVE
   tensor_scalar divide), and the old broadcast-reciprocal matmuls vanish.
 - O^T for the projection comes from PE transposes (128 bf16 rows each).
 - RMS rsqrt on DVE ((x/64)^-0.5 via tensor_scalar pow), qkv bias added in the
   DVE pipeline (per-partition scalar), v bias folded into the host-side proj
   bias (softmax rows sum to 1), so ACT runs the softmax exp ONLY.
 - RoPE elementwise work split DVE/Pool; emission order software-pipelines
   S(k+1) ahead of exp(k), stages a phase's px tiles in SBUF so each O
   qb-region accumulates contiguously (PSUM start bit stays per-element
   correct on HW), defers phase closes ~1.25 phases so early PE work (qkv+v)
   overlaps the ACT-bound exp stream, and pumps qkv/v/proj filler chunks into
   the PE gaps.

Sharding: core c handles batch c//4 and heads [3*(c%4), 3*(c%4)+3).
Each core writes a bf16 [N, C] partial; the host sums 4 partials per batch
and adds proj_bias + qkv_bias[v-part] @ proj_kernel.
"""
import sys

for _p in ("/opt/trn_rl_repo", "/opt/trn_rl_repo/concourse"):
    if _p not in sys.path:
        sys.path.insert(0, _p)

from collections import deque
from contextlib import ExitStack

import ml_dtypes
import numpy as np

import concourse.bass as bass
import concourse.mybir as mybir
import concourse.tile as tile
from concourse.bass_utils import run_bass_kernel_spmd

F32 = mybir.dt.float32
BF16 = mybir.dt.bfloat16
AF = mybir.ActivationFunctionType
ALU = mybir.AluOpType
BF = ml_dtypes.bfloat16

B, N, C = 2, 2048, 768
H, HD = 12, 64
HP = 3            # heads per core
NCORES = 8
CCH = 6           # contraction chunks of 128
NT = 4            # token tiles of 512
KB = 16           # k blocks of 128
NG = 8            # 2-kb groups per (head, qtile) phase

SWAP_MASK = [(i + 16) % 32 for i in range(32)]
PERM = np.concatenate([np.arange(0, 16), np.arange(32, 48),
                       np.arange(16, 32), np.arange(48, 64)])
SIGN = np.where(PERM < 32, -1.0, 1.0).astype(np.float32)
# rope partner of PERM-position p (SWAP_MASK's intra-32 half swap)
SWAPIDX = np.array([(p // 32) * 32 + (p + 16) % 32 for p in range(64)])

_NC_CACHE = {}


def build_nc(split_waits=True):
    nc = bass.Bass(target_bir_lowering=True)
    xT = nc.declare_dram_parameter("xT", [C, N], BF16, isOutput=False)
    wqk = nc.declare_dram_parameter("wqk", [C, HP * 128], BF16, isOutput=False)
    cosw = nc.declare_dram_parameter("cosw", [128, N], BF16, isOutput=False)
    sinw = nc.declare_dram_parameter("sinw", [128, N], BF16, isOutput=False)
    wvp = nc.declare_dram_parameter("wvp", [128, CCH * HP * 64], BF16,
                                    isOutput=False)
    wp01 = nc.declare_dram_parameter("wp01", [128, C], BF16, isOutput=False)
    wp2 = nc.declare_dram_parameter("wp2", [64, C], BF16, isOutput=False)
    # consts: [onesp(2) | sel4(512) | ident(128)]
    consts = nc.declare_dram_parameter("consts", [128, 642], BF16,
                                       isOutput=False)
    bqk = nc.declare_dram_parameter("bqk", [128, HP], F32, isOutput=False)
    out = nc.declare_dram_parameter("out", [N, C], BF16, isOutput=True)

    with tile.TileContext(nc) as tc, ExitStack() as ctx:
        sb = ctx.enter_context(tc.tile_pool(name="sb", bufs=1))
        pipe = ctx.enter_context(tc.tile_pool(name="pipe", bufs=2))
        pxp = ctx.enter_context(tc.tile_pool(name="pxp", bufs=28))
        otp = ctx.enter_context(tc.tile_pool(name="otp", bufs=4))
        pop = ctx.enter_context(tc.tile_pool(name="pop", bufs=2))
        # PSUM: 4 + 2 + 1 + 1 = 8 banks
        sp = ctx.enter_context(tc.tile_pool(name="sp", bufs=2, space="PSUM"))
        qp = ctx.enter_context(tc.tile_pool(name="qp", bufs=2, space="PSUM"))
        op = ctx.enter_context(tc.tile_pool(name="op", bufs=1, space="PSUM"))
        mp = ctx.enter_context(tc.tile_pool(name="mp", bufs=1, space="PSUM"))

        # ---------- static SBUF tiles ----------
        xs = sb.tile([128, CCH, N], BF16, tag="xs")
        wqk_sb = sb.tile([128, CCH, HP * 128], BF16, tag="wqk")
        wv_sb = sb.tile([128, CCH, HP * 64], BF16, tag="wv")
        cos_sb = sb.tile([128, N], BF16, tag="cos")
        sin_sb = sb.tile([128, N], BF16, tag="sin")
        cn = sb.tile([128, 642], BF16, tag="cn")
        onesp_sb = cn[:, 0:2]
        sel_sb = cn[:, 2:514]
        ident_sb = cn[:, 514:642]
        bqk_sb = sb.tile([128, HP], F32, tag="bqk")
        wp01_sb = sb.tile([128, C], BF16, tag="wp01")
        wp2_sb = sb.tile([64, C], BF16, tag="wp2")

        q12 = sb.tile([128, N], BF16, tag="q12")
        k12 = sb.tile([128, N], BF16, tag="k12")
        q3 = sb.tile([64, N], BF16, tag="q3")
        k3 = sb.tile([64, N], BF16, tag="k3")
        t4_all = sb.tile([128, N], BF16, tag="t4_all")
        s_sb = sb.tile([128, 512], F32, tag="s_sb")
        sv = sb.tile([128, 512], BF16, tag="sv")
        v3 = sb.tile([128, KB, HP, 65], BF16, tag="v3")
        ones48 = sb.tile([128, KB * HP], BF16, tag="ones48")
        o2 = sb.tile([128, NT, 4, 128], BF16, tag="o2")
        o1 = sb.tile([128, NT, 4, 64], BF16, tag="o1")

        def qT(h):
            return (q12[0:64], q12[64:128], q3[:])[h]

        def kT(h):
            return (k12[0:64], k12[64:128], k3[:])[h]

        # ---------- prologue DMAs (ordered for earliest qkv start) ----------
        xT_r = xT[:].rearrange("(c p) n -> p c n", p=128)
        wqk_r = wqk[:].rearrange("(c p) m -> p c m", p=128)
        d = nc.sync.dma_start
        d(cn[:], consts[:, :])
        d(bqk_sb[:], bqk[:, :])
        d(wqk_sb[:, 0:2, :], wqk_r[:, 0:2, :])
        d(xs[:, 0:3, 0:512], xT_r[:, 0:3, 0:512])      # tile-0 tokens
        d(wqk_sb[:, 2:6, :], wqk_r[:, 2:6, :])
        d(xs[:, 3:6, 0:512], xT_r[:, 3:6, 0:512])
        d(cos_sb[:, 0:1024], cosw[:, 0:1024])
        d(sin_sb[:, 0:1024], sinw[:, 0:1024])
        d(xs[:, :, 512:1024], xT_r[:, :, 512:1024])
        d(xs[:, :, 1024:1536], xT_r[:, :, 1024:1536])
        d(wv_sb[:].rearrange("p c m -> p (c m)"), wvp[:, :])
        d(xs[:, :, 1536:2048], xT_r[:, :, 1536:2048])
        d(cos_sb[:, 1024:2048], cosw[:, 1024:2048])
        d(sin_sb[:, 1024:2048], sinw[:, 1024:2048])
        d(wp01_sb[:], wp01[:, :])
        d(wp2_sb[:], wp2[:, :])

        nc.vector.memset(sv[:], 1.0)   # rows never written stay 1 (sel zeros them)
        nc.vector.memset(s_sb[:], 1.0)
        nc.vector.memset(ones48[:], 1.0)
        nc.vector.tensor_copy(
            v3[:].rearrange("p a b n -> p (a b) n", n=65)[:, :, 64], ones48[:])

        def mm(out_ap, lhsT, rhs, start, stop):
            nc.tensor.matmul(out_ap, lhsT, rhs, start=start, stop=stop,
                             skip_group_check=True)

        # ---------- qkv-head generator ----------
        # Per-tile chunks: mms -> RoPE pipe -> finA (sumsq+rsqrt) ->
        # finB (broadcast+scale). Emission defers fins so PE stays dense;
        # the qp ring (2) tolerates exactly one deferred finA.
        def qkv_gen(h):
            hs = slice(h * 128, (h + 1) * 128)
            qk = [None] * NT

            def mms(t):
                ts = slice(t * 512, (t + 1) * 512)
                qk[t] = qp.tile([128, 512], F32, tag="q", name=f"qk{t}")
                for c in range(CCH):
                    mm(qk[t][:], wqk_sb[:, c, hs], xs[:, c, ts], c == 0,
                       c == CCH - 1)

            def rope(t):
                ts = slice(t * 512, (t + 1) * 512)
                qkb = pipe.tile([128, 512], BF16, tag="qkb")
                nc.vector.tensor_scalar(qkb[:], qk[t][:], bqk_sb[:, h:h + 1],
                                        None, ALU.add)
                t1 = pipe.tile([128, 512], BF16, tag="t1")
                nc.gpsimd.tensor_mul(t1[:], qkb[:], cos_sb[:, ts])
                t2 = pipe.tile([128, 512], BF16, tag="t2")
                nc.vector.stream_shuffle(t2[:], qkb[:], SWAP_MASK)
                t3 = pipe.tile([128, 512], BF16, tag="t3")
                nc.vector.tensor_mul(t3[:], t2[:], sin_sb[:, ts])
                nc.vector.tensor_add(t4_all[:, ts], t1[:], t3[:])
                sq = pipe.tile([128, 512], BF16, tag="sq")
                if h == 0:
                    nc.vector.tensor_mul(sq[:], qkb[:], qkb[:])
                else:
                    nc.gpsimd.tensor_mul(sq[:], qkb[:], qkb[:])
                return sq

            def finA(t, sq):
                rows = slice(32 * t, 32 * t + 2)
                mm(qk[t][0:2, :], onesp_sb[:], sq[:], True, True)
                if h == 0:
                    # rsqrt = exp(-0.5 ln(ms)); same ACT table as softmax exp
                    lv = pipe.tile([2, 512], F32, tag="lv", name="lv")
                    nc.scalar.activation(lv[:], qk[t][0:2, :], AF.Ln,
                                         bias=0.0, scale=1.0 / HD)
                    nc.scalar.activation(sv[rows, :], lv[:], AF.Exp,
                                         bias=0.0, scale=-0.5)
                else:
                    nc.vector.tensor_copy(s_sb[rows, :], qk[t][0:2, :])

            def lnexp():
                lva = pipe.tile([128, 512], F32, tag="lva", name="lva")
                nc.scalar.activation(lva[:], s_sb[:], AF.Ln,
                                     bias=0.0, scale=1.0 / HD)
                nc.scalar.activation(sv[:], lva[:], AF.Exp, bias=0.0,
                                     scale=-0.5)

            def finB(t):
                ts = slice(t * 512, (t + 1) * 512)
                sqk_ps = qp.tile([128, 512], F32, tag="q")
                mm(sqk_ps[:], sel_sb[:, t * 128:(t + 1) * 128], sv[:],
                   True, True)
                sqk_sb = pipe.tile([128, 512], BF16, tag="sqk")
                nc.vector.tensor_copy(sqk_sb[:], sqk_ps[:])
                nc.vector.tensor_mul(qT(h)[:, ts], t4_all[0:64, ts],
                                     sqk_sb[0:64, :])
                nc.vector.tensor_mul(kT(h)[:, ts], t4_all[64:128, ts],
                                     sqk_sb[64:128, :])

            sqs = [None] * NT

            def do_mms(t):
                mms(t)
                sqs[t] = rope(t)

            do_mms(0)
            yield 4500
            do_mms(1)
            yield 4500
            finA(0, sqs[0])
            yield 700
            do_mms(2)
            yield 4500
            finA(1, sqs[1])
            if h == 0:
                finB(0)
            yield 1600
            finA(2, sqs[2])
            yield 700
            do_mms(3)
            yield 4500
            if h == 0:
                finB(1)
            yield 900
            finA(3, sqs[3])
            yield 700
            if h != 0:
                lnexp()
                yield 700
                finB(0)
                yield 900
                finB(1)
                yield 900
            finB(2)
            yield 900
            finB(3)
            yield 900

        # ---------- v generator ----------
        def v_gen():
            for tt in range(KB):
                v_ps = qp.tile([128, HP * 64], F32, tag="q")
                for c in range(CCH):
                    mm(v_ps[:], xs[:, c, tt * 128:(tt + 1) * 128],
                       wv_sb[:, c, :], c == 0, c == CCH - 1)
                nc.vector.tensor_copy(
                    v3[:, tt, :, 0:64],
                    v_ps[:, :].rearrange("p (h n) -> p h n", h=HP))
                yield 1500

        # ---------- proj of one (qtile, qblock) ----------
        mtr = [None]

        def proj_qb(qt, qb):
            if mtr[0] is None:
                mtr[0] = mp.tile([128, 4, 128], F32, tag="m", name="mtr")
            m = mtr[0]
            tr01 = m[:, qb, 0:64].bitcast(BF16)
            tr2 = m[0:64, qb, 64:128].bitcast(BF16)
            nc.tensor.transpose(tr01, o2[:, qt, qb, :], ident_sb[:])
            nc.tensor.transpose(tr2, o1[:, qt, qb, :], ident_sb[:])
            on_act = qt == 3   # ACT is idle once the last exps drain
            ot01 = otp.tile([128, 128], BF16, tag="ot01")
            ot2 = otp.tile([64, 128], BF16, tag="ot2")
            if on_act:
                nc.scalar.activation(ot01[:], tr01, AF.Copy, bias=0.0,
                                     scale=1.0)
                nc.vector.tensor_copy(ot2[:], tr2)
            else:
                nc.vector.tensor_copy(ot01[:], tr01)
                nc.vector.tensor_copy(ot2[:], tr2)
            po = pop.tile([128, C], BF16, tag="po")
            for half in range(2):
                cs = slice(half * 384, (half + 1) * 384)
                p_ps = qp.tile([128, 384], F32, tag="q")
                mm(p_ps[:], ot01[:], wp01_sb[:, cs], True, False)
                mm(p_ps[:], ot2[:], wp2_sb[:, cs], False, True)
                if on_act and half == 1:
                    nc.scalar.activation(po[:, cs], p_ps[:], AF.Copy,
                                         bias=0.0, scale=1.0)
                else:
                    nc.vector.tensor_copy(po[:, cs], p_ps[:])
            tb = qt * 4 + qb
            nc.sync.dma_start(out[tb * 128:(tb + 1) * 128, :], po[:])

        # ---------- filler pump ----------
        fillers = deque()
        debt = [0.0]

        def pump(budget):
            budget += debt[0]
            while budget > 0 and fillers:
                try:
                    budget -= next(fillers[0])
                except StopIteration:
                    fillers.popleft()
            debt[0] = min(budget, 3000.0)

        def ensure_done(gen):
            """Pump until `gen` has fully emitted (emission-order guard for
            cross-generator data deps)."""
            while gen in fillers:
                pump(100000)

        # ---------- attention stream ----------
        phases = [(h, qt) for h in range(HP) for qt in range(NT)]
        px_tiles = {}
        emitted = set()

        def emit_group(p, g):
            if (p, g) in emitted:
                return
            emitted.add((p, g))
            h, qt = phases[p]
            qs = slice(qt * 512, (qt + 1) * 512)
            s_ps = sp.tile([128, 1024], F32, tag="s")
            for j in range(2):
                kb = 2 * g + j
                mm(s_ps[:, j * 512:(j + 1) * 512],
                   kT(h)[:, kb * 128:(kb + 1) * 128], qT(h)[:, qs],
                   True, True)
            px = pxp.tile([128, 1024], BF16, tag="px")
            nc.scalar.activation(px[:], s_ps[:], AF.Exp, bias=0.0, scale=0.125)
            px_tiles[(p, g)] = px

        def close_gen(p):
            if p == 0:
                ensure_done(vg)   # PV reads v3; emission-order guard
            h, qt = phases[p]
            o_ps = op.tile([128, 4, 65], F32, tag="o")
            for qb in range(4):
                for g in range(NG):
                    px = px_tiles[(p, g)]
                    for j in range(2):
                        kb = 2 * g + j
                        mm(o_ps[:, qb, :],
                           px[:, j * 512 + qb * 128:j * 512 + (qb + 1) * 128],
                           v3[:, kb, h, :],
                           qb == 0 and kb == 0, kb == KB - 1)
                if qb == 1 or qb == 3:
                    yield
            # normalize by the ones-column denominators (batched reciprocal,
            # then per-qb per-partition multiply); epilogues after ALL PV so
            # coarse WAR tracking can't serialize the qb bundles
            rec4 = pipe.tile([128, 4], F32, tag="rec4", name="rec4")
            nc.vector.reciprocal(rec4[:], o_ps[:, :, 64])
            for qb in range(4):
                dst = (o2[:, qt, qb, h * 64:(h + 1) * 64] if h < 2
                       else o1[:, qt, qb, :])
                nc.vector.tensor_scalar(dst, o_ps[:, qb, 0:64],
                                        rec4[:, qb:qb + 1], None, ALU.mult)
                if h == 2:
                    proj_qb(qt, qb)
                yield
            for g in range(NG):
                del px_tiles[(p, g)]

        def drain_close(cg):
            for _ in cg:
                pass

        # ---------- main schedule ----------
        # Phase 0 runs with qkv(h0) inlined per tile: tile t unlocks S groups
        # 2t, 2t+1 (k-tiles) while qT(qt0) comes entirely from tile 0.
        qg0 = qkv_gen(0)
        vg = v_gen()
        g1, g2 = qkv_gen(1), qkv_gen(2)
        fillers.append(vg)
        fillers.append(g1)
        need_gen = {1: g1, 2: g2}

        def drain_n(gen, n_chunks):
            for _ in range(n_chunks):
                next(gen)

        # phase 0: dense qkv-h0 mm front with S groups as tiles finish
        drain_n(qg0, 5)              # thru finB(0): groups 0-1 ready
        for g in range(NG):
            if g == 2:
                drain_n(qg0, 3)      # finA2, mms3, finB1
            elif g == 4:
                drain_n(qg0, 2)      # finA3, finB2
            elif g == 6:
                drain_n(qg0, 1)      # finB3
            emit_group(0, g)
            if g >= 5:
                emit_group(1, g - 5)   # pre-emit ahead of pumped v backlog
            pump(600)

        # Steady state: early closes deferred ~2 phases (px ring holds ~3
        # phases) so v/qkv fillers use the early PE slack; late closes pulled
        # in so proj work overlaps the remaining exp stream.
        NP = len(phases)
        close_at = {}
        for p in range(NP - 1):
            # early closes deferred 3 phases (shifts PV out of the PE-heavy
            # qkv/v window); h2 closes pulled in so proj overlaps exps
            lag = (p + 3, 1) if p < 8 else (p + 1, 0)
            close_at.setdefault(lag, []).append(p)

        def after_close(cp):
            if phases[cp] == (0, 2):
                fillers.append(g2)

        active_closes = []
        for p in range(1, NP):
            nh = phases[p][0]
            if nh != phases[p - 1][0]:
                ensure_done(need_gen[nh])
            for g in range(NG):
                emit_group(p, g)
                pump(1800 if p <= 4 else 1500)
                for cp in close_at.get((p, g), []):
                    active_closes.append((cp, close_gen(cp)))
                if active_closes:
                    cp, cg = active_closes[0]
                    try:
                        next(cg)
                    except StopIteration:
                        active_closes.pop(0)
                        after_close(cp)
        for cp, cg in active_closes:
            drain_close(cg)
            after_close(cp)
        drain_close(close_gen(NP - 1))
        while fillers:
            pump(100000)

    if split_waits:
        _split_waits(nc)
    return nc


def _split_waits(nc):
    """Walrus lowers at most one sync-wait per instruction; move excess waits
    onto NoOps inserted just before, on the same engine queue."""
    k = 0
    for fn in nc.m.functions:
        for bb in fn.blocks:
            il = bb.instructions
            idx = 0
            while idx < len(il):
                inst = il[idx]
                si = inst.sync_info
                eng = getattr(inst, "engine", None)
                if (si is not None and len(si.on_wait) > 1
                        and eng is not None
                        and str(eng) != "EngineType.Unassigned"):
                    waits = list(si.on_wait)
                    inst.sync_info = mybir.SyncInfo(
                        on_wait=[waits[-1]], on_update=list(si.on_update))
                    for w in waits[:-1]:
                        nop = mybir.InstNoOp(
                            name=f"I-waitnop-{k}", engine=eng, ins=[], outs=[],
                            sync_info=mybir.SyncInfo(on_wait=[w], on_update=[]))
                        k += 1
                        il.insert(idx, nop)
                        idx += 1
                idx += 1


def _prep_core_inputs(core, x, rope_cos, rope_sin, qkv_kernel, qkv_bias,
                      proj_kernel, proj_bias, q_norm_w, k_norm_w):
    b = core // 4
    heads = [3 * (core % 4) + i for i in range(HP)]

    wq = qkv_kernel.reshape(C, 3, H, HD)
    bq = qkv_bias.reshape(3, H, HD)

    xTa = np.ascontiguousarray(x[b].T).astype(BF)

    wqk = np.empty((C, HP * 128), np.float32)
    bqk = np.zeros((128, HP), np.float32)
    for i, h in enumerate(heads):
        wqk[:, i * 128:i * 128 + 64] = wq[:, 0, h, PERM]
        wqk[:, i * 128 + 64:(i + 1) * 128] = wq[:, 1, h, PERM]
        bqk[0:64, i] = bq[0, h, PERM]
        bqk[64:128, i] = bq[1, h, PERM]

    wv = np.zeros((C, HP * 64), np.float32)
    for i, h in enumerate(heads):
        wv[:, i * 64:(i + 1) * 64] = wq[:, 2, h, :]
    # packed [p, (c m)] so the SBUF copy is one dense DMA
    wvp = wv.reshape(CCH, 128, HP * 64).transpose(1, 0, 2).reshape(128, -1)

    cosT = rope_cos.T  # (HD, N)
    sinT = rope_sin.T
    cosw = np.empty((128, N), np.float32)
    sinw = np.empty((128, N), np.float32)
    cosw[0:64] = cosT[PERM] * q_norm_w[PERM][:, None]
    cosw[64:128] = cosT[PERM] * k_norm_w[PERM][:, None]
    # sin multiplies the SHUFFLED (partner) value -> partner's norm weight
    qn_p = q_norm_w[PERM][SWAPIDX]
    kn_p = k_norm_w[PERM][SWAPIDX]
    sinw[0:64] = SIGN[:, None] * sinT[PERM] * qn_p[:, None]
    sinw[64:128] = SIGN[:, None] * sinT[PERM] * kn_p[:, None]

    onesp = np.zeros((128, 2), np.float32)
    onesp[0:64, 0] = 1.0
    onesp[64:128, 1] = 1.0

    sel4 = np.zeros((128, 512), np.float32)
    for t in range(NT):
        sel4[32 * t, t * 128:t * 128 + 64] = 1.0
        sel4[32 * t + 1, t * 128 + 64:(t + 1) * 128] = 1.0

    rows01 = np.concatenate([np.arange(h * HD, (h + 1) * HD)
                             for h in heads[0:2]])
    rows2 = np.arange(heads[2] * HD, (heads[2] + 1) * HD)
    wp01 = proj_kernel[rows01, :]
    wp2 = proj_kernel[rows2, :]

    consts = np.zeros((128, 642), np.float32)
    consts[:, 0:2] = onesp
    consts[:, 2:514] = sel4
    consts[:, 514:642] = np.eye(128, dtype=np.float32)
    return {"xT": xTa, "wqk": wqk.astype(BF), "bqk": bqk,
            "cosw": cosw.astype(BF), "sinw": sinw.astype(BF),
            "wvp": np.ascontiguousarray(wvp).astype(BF),
            "wp01": np.ascontiguousarray(wp01).astype(BF),
            "wp2": np.ascontiguousarray(wp2).astype(BF),
            "consts": consts.astype(BF)}


def kernel(x, rope_cos, rope_sin, qkv_kernel, qkv_bias, proj_kernel,
           proj_bias, q_norm_w, k_norm_w, _trace=False):
    args = [np.asarray(a, dtype=np.float32) for a in
            (x, rope_cos, rope_sin, qkv_kernel, qkv_bias, proj_kernel,
             proj_bias, q_norm_w, k_norm_w)]
    in_maps = [_prep_core_inputs(c, *args) for c in range(NCORES)]

    if "nc" not in _NC_CACHE:
        _NC_CACHE["nc"] = build_nc()
    nc = _NC_CACHE["nc"]

    res = run_bass_kernel_spmd(nc, in_maps, core_ids=list(range(NCORES)),
                               trace=_trace)
    parts = [np.asarray(res.results[c]["out"]).astype(np.float32)
             for c in range(NCORES)]
    # v-bias contributes exactly bv @ proj_kernel (softmax rows sum to 1)
    pb = (np.asarray(proj_bias, dtype=np.float32)
          + np.asarray(qkv_bias, dtype=np.float32)[2 * C:]
          @ np.asarray(proj_kernel, dtype=np.float32))
    out = np.empty((B, N, C), np.float32)
    for b in range(B):
        out[b] = (parts[4 * b] + parts[4 * b + 1] + parts[4 * b + 2]
                  + parts[4 * b + 3] + pb)
    if _trace:
        kernel.last_results = res
    return out


# revision 4
# speedup vs baseline: 1.5609x; 1.0209x over previous
"""Multi-head attention (RMSNorm-QK + RoPE + softmax + proj) on 8 Trainium2 cores.

v2 design (cost-model-driven rewrite of the baseline):
 - bf16 operands everywhere (matmuls cost 1 cyc/row like fp32r, but DVE gets
   2x modes and DMA halves); fp32 PSUM accumulation throughout.
 - Transposed PV: O tiles are [128 q, 65] (64 dims + ones col for the softmax
   denominator), using all 128 output partitions -> PV drops from 32768 to
   16640 cyc/head, the denominator becomes a per-partition column (one DVE
   tensor_scalar divide), and the old broadcast-reciprocal matmuls vanish.
 - O^T for the projection comes from PE transposes (128 bf16 rows each).
 - RMS rsqrt on DVE ((x/64)^-0.5 via tensor_scalar pow), qkv bias added in the
   DVE pipeline (per-partition scalar), v bias folded into the host-side proj
   bias (softmax rows sum to 1), so ACT runs the softmax exp ONLY.
 - RoPE elementwise work split DVE/Pool; emission order software-pipelines
   S(k+1) ahead of exp(k), stages a phase's px tiles in SBUF so each O
   qb-region accumulates contiguously (PSUM start bit stays per-element
   correct on HW), defers phase closes ~1.25 phases so early PE work (qkv+v)
   overlaps the ACT-bound exp stream, and pumps qkv/v/proj filler chunks into
   the PE gaps.

Sharding: core c handles batch c//4 and heads [3*(c%4), 3*(c%4)+3).
Each core writes a bf16 [N, C] partial; the host sums 4 partials per batch
and adds proj_bias + qkv_bias[v-part] @ proj_kernel.
"""
import sys

for _p in ("/opt/trn_rl_repo", "/opt/trn_rl_repo/concourse"):
    if _p not in sys.path:
        sys.path.insert(0, _p)

from collections import deque
from contextlib import ExitStack

import ml_dtypes
import numpy as np

import concourse.bass as bass
import concourse.mybir as mybir
import concourse.tile as tile
from concourse.bass_utils import run_bass_kernel_spmd

F32 = mybir.dt.float32
BF16 = mybir.dt.bfloat16
AF = mybir.ActivationFunctionType
ALU = mybir.AluOpType
BF = ml_dtypes.bfloat16

B, N, C = 2, 2048, 768
H, HD = 12, 64
HP = 3            # heads per core
NCORES = 8
CCH = 6           # contraction chunks of 128
NT = 4            # token tiles of 512
KB = 16           # k blocks of 128
NG = 8            # 2-kb groups per (head, qtile) phase

SWAP_MASK = [(i + 16) % 32 for i in range(32)]
PERM = np.concatenate([np.arange(0, 16), np.arange(32, 48),
                       np.arange(16, 32), np.arange(48, 64)])
SIGN = np.where(PERM < 32, -1.0, 1.0).astype(np.float32)
# rope partner of PERM-position p (SWAP_MASK's intra-32 half swap)
SWAPIDX = np.array([(p // 32) * 32 + (p + 16) % 32 for p in range(64)])

_NC_CACHE = {}


def build_nc(split_waits=True):
    nc = bass.Bass(target_bir_lowering=True)
    xT = nc.declare_dram_parameter("xT", [C, N], BF16, isOutput=False)
    wqk = nc.declare_dram_parameter("wqk", [C, HP * 128], BF16, isOutput=False)
    cosw = nc.declare_dram_parameter("cosw", [128, N], BF16, isOutput=False)
    sinw = nc.declare_dram_parameter("sinw", [128, N], BF16, isOutput=False)
    wvp = nc.declare_dram_parameter("wvp", [128, CCH * HP * 64], BF16,
                                    isOutput=False)
    wp01 = nc.declare_dram_parameter("wp01", [128, C], BF16, isOutput=False)
    wp2 = nc.declare_dram_parameter("wp2", [64, C], BF16, isOutput=False)
    # consts: [onesp(2) | sel4(512) | ident(128)]
    consts = nc.declare_dram_parameter("consts", [128, 642], BF16,
                                       isOutput=False)
    bqk = nc.declare_dram_parameter("bqk", [128, HP], F32, isOutput=False)
    out = nc.declare_dram_parameter("out", [N, C], BF16, isOutput=True)

    with tile.TileContext(nc) as tc, ExitStack() as ctx:
        sb = ctx.enter_context(tc.tile_pool(name="sb", bufs=1))
        pipe = ctx.enter_context(tc.tile_pool(name="pipe", bufs=2))
        pxp = ctx.enter_context(tc.tile_pool(name="pxp", bufs=28))
        otp = ctx.enter_context(tc.tile_pool(name="otp", bufs=4))
        pop = ctx.enter_context(tc.tile_pool(name="pop", bufs=2))
        # PSUM: 4 + 2 + 1 + 1 = 8 banks
        sp = ctx.enter_context(tc.tile_pool(name="sp", bufs=2, space="PSUM"))
        qp = ctx.enter_context(tc.tile_pool(name="qp", bufs=2, space="PSUM"))
        op = ctx.enter_context(tc.tile_pool(name="op", bufs=1, space="PSUM"))
        mp = ctx.enter_context(tc.tile_pool(name="mp", bufs=1, space="PSUM"))

        # ---------- static SBUF tiles ----------
        xs = sb.tile([128, CCH, N], BF16, tag="xs")
        wqk_sb = sb.tile([128, CCH, HP * 128], BF16, tag="wqk")
        wv_sb = sb.tile([128, CCH, HP * 64], BF16, tag="wv")
        cos_sb = sb.tile([128, N], BF16, tag="cos")
        sin_sb = sb.tile([128, N], BF16, tag="sin")
        cn = sb.tile([128, 642], BF16, tag="cn")
        onesp_sb = cn[:, 0:2]
        sel_sb = cn[:, 2:514]
        ident_sb = cn[:, 514:642]
        bqk_sb = sb.tile([128, HP], F32, tag="bqk")
        wp01_sb = sb.tile([128, C], BF16, tag="wp01")
        wp2_sb = sb.tile([64, C], BF16, tag="wp2")

        q12 = sb.tile([128, N], BF16, tag="q12")
        k12 = sb.tile([128, N], BF16, tag="k12")
        q3 = sb.tile([64, N], BF16, tag="q3")
        k3 = sb.tile([64, N], BF16, tag="k3")
        t4_all = sb.tile([128, N], BF16, tag="t4_all")
        s_sb = sb.tile([128, 512], F32, tag="s_sb")
        sv = sb.tile([128, 512], BF16, tag="sv")
        v3 = sb.tile([128, KB, HP, 65], BF16, tag="v3")
        ones48 = sb.tile([128, KB * HP], BF16, tag="ones48")
        o2 = sb.tile([128, NT, 4, 128], BF16, tag="o2")
        o1 = sb.tile([128, NT, 4, 64], BF16, tag="o1")

        def qT(h):
            return (q12[0:64], q12[64:128], q3[:])[h]

        def kT(h):
            return (k12[0:64], k12[64:128], k3[:])[h]

        # ---------- prologue DMAs (ordered for earliest qkv start) ----------
        xT_r = xT[:].rearrange("(c p) n -> p c n", p=128)
        wqk_r = wqk[:].rearrange("(c p) m -> p c m", p=128)
        d = nc.sync.dma_start
        d(cn[:], consts[:, :])
        d(bqk_sb[:], bqk[:, :])
        d(wqk_sb[:, 0:2, :], wqk_r[:, 0:2, :])
        d(xs[:, 0:3, 0:512], xT_r[:, 0:3, 0:512])      # tile-0 tokens
        d(wqk_sb[:, 2:6, :], wqk_r[:, 2:6, :])
        d(xs[:, 3:6, 0:512], xT_r[:, 3:6, 0:512])
        d(cos_sb[:, 0:1024], cosw[:, 0:1024])
        d(sin_sb[:, 0:1024], sinw[:, 0:1024])
        d(xs[:, :, 512:1024], xT_r[:, :, 512:1024])
        d(xs[:, :, 1024:1536], xT_r[:, :, 1024:1536])
        d(wv_sb[:].rearrange("p c m -> p (c m)"), wvp[:, :])
        d(xs[:, :, 1536:2048], xT_r[:, :, 1536:2048])
        d(cos_sb[:, 1024:2048], cosw[:, 1024:2048])
        d(sin_sb[:, 1024:2048], sinw[:, 1024:2048])
        d(wp01_sb[:], wp01[:, :])
        d(wp2_sb[:], wp2[:, :])

        nc.vector.memset(sv[:], 1.0)   # rows never written stay 1 (sel zeros them)
        nc.vector.memset(s_sb[:], 1.0)
        nc.vector.memset(ones48[:], 1.0)
        nc.vector.tensor_copy(
            v3[:].rearrange("p a b n -> p (a b) n", n=65)[:, :, 64], ones48[:])

        def mm(out_ap, lhsT, rhs, start, stop):
            nc.tensor.matmul(out_ap, lhsT, rhs, start=start, stop=stop,
                             skip_group_check=True)

        # ---------- qkv-head generator ----------
        # Per-tile chunks: mms -> RoPE pipe -> finA (sumsq+rsqrt) ->
        # finB (broadcast+scale). Emission defers fins so PE stays dense;
        # the qp ring (2) tolerates exactly one deferred finA.
        def qkv_gen(h):
            hs = slice(h * 128, (h + 1) * 128)
            qk = [None] * NT

            def mms(t):
                ts = slice(t * 512, (t + 1) * 512)
                qk[t] = qp.tile([128, 512], F32, tag="q", name=f"qk{t}")
                for c in range(CCH):
                    mm(qk[t][:], wqk_sb[:, c, hs], xs[:, c, ts], c == 0,
                       c == CCH - 1)

            def rope(t):
                ts = slice(t * 512, (t + 1) * 512)
                qkb = pipe.tile([128, 512], BF16, tag="qkb")
                nc.vector.tensor_scalar(qkb[:], qk[t][:], bqk_sb[:, h:h + 1],
                                        None, ALU.add)
                t1 = pipe.tile([128, 512], BF16, tag="t1")
                nc.gpsimd.tensor_mul(t1[:], qkb[:], cos_sb[:, ts])
                t2 = pipe.tile([128, 512], BF16, tag="t2")
                nc.vector.stream_shuffle(t2[:], qkb[:], SWAP_MASK)
                t3 = pipe.tile([128, 512], BF16, tag="t3")
                nc.vector.tensor_mul(t3[:], t2[:], sin_sb[:, ts])
                nc.vector.tensor_add(t4_all[:, ts], t1[:], t3[:])
                sq = pipe.tile([128, 512], BF16, tag="sq")
                if h == 0:
                    nc.vector.tensor_mul(sq[:], qkb[:], qkb[:])
                else:
                    nc.gpsimd.tensor_mul(sq[:], qkb[:], qkb[:])
                return sq

            def finA(t, sq):
                rows = slice(32 * t, 32 * t + 2)
                mm(qk[t][0:2, :], onesp_sb[:], sq[:], True, True)
                if h == 0:
                    # rsqrt = exp(-0.5 ln(ms)); same ACT table as softmax exp
                    lv = pipe.tile([2, 512], F32, tag="lv", name="lv")
                    nc.scalar.activation(lv[:], qk[t][0:2, :], AF.Ln,
                                         bias=0.0, scale=1.0 / HD)
                    nc.scalar.activation(sv[rows, :], lv[:], AF.Exp,
                                         bias=0.0, scale=-0.5)
                else:
                    nc.vector.tensor_copy(s_sb[rows, :], qk[t][0:2, :])

            def lnexp():
                lva = pipe.tile([128, 512], F32, tag="lva", name="lva")
                nc.scalar.activation(lva[:], s_sb[:], AF.Ln,
                                     bias=0.0, scale=1.0 / HD)
                nc.scalar.activation(sv[:], lva[:], AF.Exp, bias=0.0,
                                     scale=-0.5)

            def finB(t):
                ts = slice(t * 512, (t + 1) * 512)
                sqk_ps = qp.tile([128, 512], F32, tag="q")
                mm(sqk_ps[:], sel_sb[:, t * 128:(t + 1) * 128], sv[:],
                   True, True)
                sqk_sb = pipe.tile([128, 512], BF16, tag="sqk")
                nc.vector.tensor_copy(sqk_sb[:], sqk_ps[:])
                nc.vector.tensor_mul(qT(h)[:, ts], t4_all[0:64, ts],
                                     sqk_sb[0:64, :])
                nc.vector.tensor_mul(kT(h)[:, ts], t4_all[64:128, ts],
                                     sqk_sb[64:128, :])

            sqs = [None] * NT

            def do_mms(t):
                mms(t)
                sqs[t] = rope(t)

            do_mms(0)
            yield 4500
            do_mms(1)
            yield 4500
            finA(0, sqs[0])
            yield 700
            do_mms(2)
            yield 4500
            finA(1, sqs[1])
            if h == 0:
                finB(0)
            yield 1600
            finA(2, sqs[2])
            yield 700
            do_mms(3)
            yield 4500
            if h == 0:
                finB(1)
            yield 900
            finA(3, sqs[3])
            yield 700
            if h != 0:
                lnexp()
                yield 700
                finB(0)
                yield 900
                finB(1)
                yield 900
            finB(2)
            yield 900
            finB(3)
            yield 900

        # ---------- v generator ----------
        def v_gen():
            for tt in range(KB):
                v_ps = qp.tile([128, HP * 64], F32, tag="q")
                for c in range(CCH):
                    mm(v_ps[:], xs[:, c, tt * 128:(tt + 1) * 128],
                       wv_sb[:, c, :], c == 0, c == CCH - 1)
                nc.vector.tensor_copy(
                    v3[:, tt, :, 0:64],
                    v_ps[:, :].rearrange("p (h n) -> p h n", h=HP))
                yield 1500

        # ---------- proj of one (qtile, qblock) ----------
        mtr = [None]

        def proj_qb(qt, qb):
            if mtr[0] is None:
                mtr[0] = mp.tile([128, 4, 128], F32, tag="m", name="mtr")
            m = mtr[0]
            tr01 = m[:, qb, 0:64].bitcast(BF16)
            tr2 = m[0:64, qb, 64:128].bitcast(BF16)
            nc.tensor.transpose(tr01, o2[:, qt, qb, :], ident_sb[:])
            nc.tensor.transpose(tr2, o1[:, qt, qb, :], ident_sb[:])
            on_act = qt == 3   # ACT is idle once the last exps drain
            ot01 = otp.tile([128, 128], BF16, tag="ot01")
            ot2 = otp.tile([64, 128], BF16, tag="ot2")
            if on_act:
                nc.scalar.activation(ot01[:], tr01, AF.Copy, bias=0.0,
                                     scale=1.0)
                nc.vector.tensor_copy(ot2[:], tr2)
            else:
                nc.vector.tensor_copy(ot01[:], tr01)
                nc.vector.tensor_copy(ot2[:], tr2)
            po = pop.tile([128, C], BF16, tag="po")
            for half in range(2):
                cs = slice(half * 384, (half + 1) * 384)
                p_ps = qp.tile([128, 384], F32, tag="q")
                mm(p_ps[:], ot01[:], wp01_sb[:, cs], True, False)
                mm(p_ps[:], ot2[:], wp2_sb[:, cs], False, True)
                if on_act and half == 1:
                    nc.scalar.activation(po[:, cs], p_ps[:], AF.Copy,
                                         bias=0.0, scale=1.0)
                else:
                    nc.vector.tensor_copy(po[:, cs], p_ps[:])
            tb = qt * 4 + qb
            nc.sync.dma_start(out[tb * 128:(tb + 1) * 128, :], po[:])

        # ---------- filler pump ----------
        fillers = deque()
        debt = [0.0]

        def pump(budget):
            budget += debt[0]
            while budget > 0 and fillers:
                try:
                    budget -= next(fillers[0])
                except StopIteration:
                    fillers.popleft()
            debt[0] = min(budget, 3000.0)

        def ensure_done(gen):
            """Pump until `gen` has fully emitted (emission-order guard for
            cross-generator data deps)."""
            while gen in fillers:
                pump(100000)

        # ---------- attention stream ----------
        phases = [(h, qt) for h in range(HP) for qt in range(NT)]
        px_tiles = {}
        emitted = set()

        def emit_group(p, g):
            if (p, g) in emitted:
                return
            emitted.add((p, g))
            h, qt = phases[p]
            qs = slice(qt * 512, (qt + 1) * 512)
            s_ps = sp.tile([128, 1024], F32, tag="s")
            for j in range(2):
                kb = 2 * g + j
                mm(s_ps[:, j * 512:(j + 1) * 512],
                   kT(h)[:, kb * 128:(kb + 1) * 128], qT(h)[:, qs],
                   True, True)
            px = pxp.tile([128, 1024], BF16, tag="px")
            nc.scalar.activation(px[:], s_ps[:], AF.Exp, bias=0.0, scale=0.125)
            px_tiles[(p, g)] = px

        def close_gen(p):
            if p == 0:
                ensure_done(vg)   # PV reads v3; emission-order guard
            h, qt = phases[p]
            o_ps = op.tile([128, 4, 65], F32, tag="o")
            for qb in range(4):
                for g in range(NG):
                    px = px_tiles[(p, g)]
                    for j in range(2):
                        kb = 2 * g + j
                        mm(o_ps[:, qb, :],
                           px[:, j * 512 + qb * 128:j * 512 + (qb + 1) * 128],
                           v3[:, kb, h, :],
                           qb == 0 and kb == 0, kb == KB - 1)
                if qb == 1 or qb == 3:
                    yield
            # normalize by the ones-column denominators (batched reciprocal,
            # then per-qb per-partition multiply); epilogues after ALL PV so
            # coarse WAR tracking can't serialize the qb bundles
            rec4 = pipe.tile([128, 4], F32, tag="rec4", name="rec4")
            nc.vector.reciprocal(rec4[:], o_ps[:, :, 64])
            for qb in range(4):
                dst = (o2[:, qt, qb, h * 64:(h + 1) * 64] if h < 2
                       else o1[:, qt, qb, :])
                nc.vector.tensor_scalar(dst, o_ps[:, qb, 0:64],
                                        rec4[:, qb:qb + 1], None, ALU.mult)
                if h == 2:
                    proj_qb(qt, qb)
                yield
            for g in range(NG):
                del px_tiles[(p, g)]

        def drain_close(cg):
            for _ in cg:
                pass

        # ---------- main schedule ----------
        # Phase 0 runs with qkv(h0) inlined per tile: tile t unlocks S groups
        # 2t, 2t+1 (k-tiles) while qT(qt0) comes entirely from tile 0.
        qg0 = qkv_gen(0)
        vg = v_gen()
        g1, g2 = qkv_gen(1), qkv_gen(2)
        fillers.append(vg)
        fillers.append(g1)
        need_gen = {1: g1, 2: g2}

        def drain_n(gen, n_chunks):
            for _ in range(n_chunks):
                next(gen)

        # phase 0: dense qkv-h0 mm front with S groups as tiles finish
        drain_n(qg0, 5)              # thru finB(0): groups 0-1 ready
        for g in range(NG):
            if g == 2:
                drain_n(qg0, 3)      # finA2, mms3, finB1
            elif g == 4:
                drain_n(qg0, 2)      # finA3, finB2
            elif g == 6:
                drain_n(qg0, 1)      # finB3
            emit_group(0, g)
            if g >= 5:
                emit_group(1, g - 5)   # pre-emit ahead of pumped v backlog
            pump(600)

        # Steady state: early closes deferred ~2 phases (px ring holds ~3
        # phases) so v/qkv fillers use the early PE slack; late closes pulled
        # in so proj work overlaps the remaining exp stream.
        NP = len(phases)
        close_at = {}
        for p in range(NP - 1):
            # early closes deferred 3 phases (shifts PV out of the PE-heavy
            # qkv/v window); h2 closes pulled in so proj overlaps exps
            lag = (p + 3, 1) if p < 8 else (p + 1, 0)
            close_at.setdefault(lag, []).append(p)

        def after_close(cp):
            if phases[cp] == (0, 2):
                fillers.append(g2)

        active_closes = []
        for p in range(1, NP):
            nh = phases[p][0]
            if nh != phases[p - 1][0]:
                ensure_done(need_gen[nh])
            for g in range(NG):
                emit_group(p, g)
                pump(2400 if p <= 4 else 2200)
                for cp in close_at.get((p, g), []):
                    active_closes.append((cp, close_gen(cp)))
                if active_closes:
                    cp, cg = active_closes[0]
                    try:
                        next(cg)
                    except StopIteration:
                        active_closes.pop(0)
                        after_close(cp)
        for cp, cg in active_closes:
            drain_close(cg)
            after_close(cp)
        drain_close(close_gen(NP - 1))
        while fillers:
            pump(100000)

    if split_waits:
        _split_waits(nc)
    return nc


def _split_waits(nc):
    """Walrus lowers at most one sync-wait per instruction; move excess waits
    onto NoOps inserted just before, on the same engine queue."""
    k = 0
    for fn in nc.m.functions:
        for bb in fn.blocks:
            il = bb.instructions
            idx = 0
            while idx < len(il):
                inst = il[idx]
                si = inst.sync_info
                eng = getattr(inst, "engine", None)
                if (si is not None and len(si.on_wait) > 1
                        and eng is not None
                        and str(eng) != "EngineType.Unassigned"):
                    waits = list(si.on_wait)
                    inst.sync_info = mybir.SyncInfo(
                        on_wait=[waits[-1]], on_update=list(si.on_update))
                    for w in waits[:-1]:
                        nop = mybir.InstNoOp(
                            name=f"I-waitnop-{k}", engine=eng, ins=[], outs=[],
                            sync_info=mybir.SyncInfo(on_wait=[w], on_update=[]))
                        k += 1
                        il.insert(idx, nop)
                        idx += 1
                idx += 1


def _prep_core_inputs(core, x, rope_cos, rope_sin, qkv_kernel, qkv_bias,
                      proj_kernel, proj_bias, q_norm_w, k_norm_w):
    b = core // 4
    heads = [3 * (core % 4) + i for i in range(HP)]

    wq = qkv_kernel.reshape(C, 3, H, HD)
    bq = qkv_bias.reshape(3, H, HD)

    xTa = np.ascontiguousarray(x[b].T).astype(BF)

    wqk = np.empty((C, HP * 128), np.float32)
    bqk = np.zeros((128, HP), np.float32)
    for i, h in enumerate(heads):
        wqk[:, i * 128:i * 128 + 64] = wq[:, 0, h, PERM]
        wqk[:, i * 128 + 64:(i + 1) * 128] = wq[:, 1, h, PERM]
        bqk[0:64, i] = bq[0, h, PERM]
        bqk[64:128, i] = bq[1, h, PERM]

    wv = np.zeros((C, HP * 64), np.float32)
    for i, h in enumerate(heads):
        wv[:, i * 64:(i + 1) * 64] = wq[:, 2, h, :]
    # packed [p, (c m)] so the SBUF copy is one dense DMA
    wvp = wv.reshape(CCH, 128, HP * 64).transpose(1, 0, 2).reshape(128, -1)

    cosT = rope_cos.T  # (HD, N)
    sinT = rope_sin.T
    cosw = np.empty((128, N), np.float32)
    sinw = np.empty((128, N), np.float32)
    cosw[0:64] = cosT[PERM] * q_norm_w[PERM][:, None]
    cosw[64:128] = cosT[PERM] * k_norm_w[PERM][:, None]
    # sin multiplies the SHUFFLED (partner) value -> partner's norm weight
    qn_p = q_norm_w[PERM][SWAPIDX]
    kn_p = k_norm_w[PERM][SWAPIDX]
    sinw[0:64] = SIGN[:, None] * sinT[PERM] * qn_p[:, None]
    sinw[64:128] = SIGN[:, None] * sinT[PERM] * kn_p[:, None]

    onesp = np.zeros((128, 2), np.float32)
    onesp[0:64, 0] = 1.0
    onesp[64:128, 1] = 1.0

    sel4 = np.zeros((128, 512), np.float32)
    for t in range(NT):
        sel4[32 * t, t * 128:t * 128 + 64] = 1.0
        sel4[32 * t + 1, t * 128 + 64:(t + 1) * 128] = 1.0

    rows01 = np.concatenate([np.arange(h * HD, (h + 1) * HD)
                             for h in heads[0:2]])
    rows2 = np.arange(heads[2] * HD, (heads[2] + 1) * HD)
    wp01 = proj_kernel[rows01, :]
    wp2 = proj_kernel[rows2, :]

    consts = np.zeros((128, 642), np.float32)
    consts[:, 0:2] = onesp
    consts[:, 2:514] = sel4
    consts[:, 514:642] = np.eye(128, dtype=np.float32)
    return {"xT": xTa, "wqk": wqk.astype(BF), "bqk": bqk,
            "cosw": cosw.astype(BF), "sinw": sinw.astype(BF),
            "wvp": np.ascontiguousarray(wvp).astype(BF),
            "wp01": np.ascontiguousarray(wp01).astype(BF),
            "wp2": np.ascontiguousarray(wp2).astype(BF),
            "consts": consts.astype(BF)}


def kernel(x, rope_cos, rope_sin, qkv_kernel, qkv_bias, proj_kernel,
           proj_bias, q_norm_w, k_norm_w, _trace=False):
    args = [np.asarray(a, dtype=np.float32) for a in
            (x, rope_cos, rope_sin, qkv_kernel, qkv_bias, proj_kernel,
             proj_bias, q_norm_w, k_norm_w)]
    in_maps = [_prep_core_inputs(c, *args) for c in range(NCORES)]

    if "nc" not in _NC_CACHE:
        _NC_CACHE["nc"] = build_nc()
    nc = _NC_CACHE["nc"]

    res = run_bass_kernel_spmd(nc, in_maps, core_ids=list(range(NCORES)),
                               trace=_trace)
    parts = [np.asarray(res.results[c]["out"]).astype(np.float32)
             for c in range(NCORES)]
    # v-bias contributes exactly bv @ proj_kernel (softmax rows sum to 1)
    pb = (np.asarray(proj_bias, dtype=np.float32)
          + np.asarray(qkv_bias, dtype=np.float32)[2 * C:]
          @ np.asarray(proj_kernel, dtype=np.float32))
    out = np.empty((B, N, C), np.float32)
    for b in range(B):
        out[b] = (parts[4 * b] + parts[4 * b + 1] + parts[4 * b + 2]
                  + parts[4 * b + 3] + pb)
    if _trace:
        kernel.last_results = res
    return out


# revision 5
# speedup vs baseline: 1.6017x; 1.0261x over previous
"""Multi-head attention (RMSNorm-QK + RoPE + softmax + proj) on 8 Trainium2 cores.

v2 design (cost-model-driven rewrite of the baseline):
 - bf16 operands everywhere (matmuls cost 1 cyc/row like fp32r, but DVE gets
   2x modes and DMA halves); fp32 PSUM accumulation throughout.
 - Transposed PV: O tiles are [128 q, 65] (64 dims + ones col for the softmax
   denominator), using all 128 output partitions -> PV drops from 32768 to
   16640 cyc/head, the denominator becomes a per-partition column (one DVE
   tensor_scalar divide), and the old broadcast-reciprocal matmuls vanish.
 - O^T for the projection comes from PE transposes (128 bf16 rows each).
 - RMS rsqrt on DVE ((x/64)^-0.5 via tensor_scalar pow), qkv bias added in the
   DVE pipeline (per-partition scalar), v bias folded into the host-side proj
   bias (softmax rows sum to 1), so ACT runs the softmax exp ONLY.
 - RoPE elementwise work split DVE/Pool; emission order software-pipelines
   S(k+1) ahead of exp(k), stages a phase's px tiles in SBUF so each O
   qb-region accumulates contiguously (PSUM start bit stays per-element
   correct on HW), defers phase closes ~1.25 phases so early PE work (qkv+v)
   overlaps the ACT-bound exp stream, and pumps qkv/v/proj filler chunks into
   the PE gaps.

Sharding: core c handles batch c//4 and heads [3*(c%4), 3*(c%4)+3).
Each core writes a bf16 [N, C] partial; the host sums 4 partials per batch
and adds proj_bias + qkv_bias[v-part] @ proj_kernel.
"""
import sys

for _p in ("/opt/trn_rl_repo", "/opt/trn_rl_repo/concourse"):
    if _p not in sys.path:
        sys.path.insert(0, _p)

from collections import deque
from contextlib import ExitStack

import ml_dtypes
import numpy as np

import concourse.bass as bass
import concourse.mybir as mybir
import concourse.tile as tile
from concourse.bass_utils import run_bass_kernel_spmd

F32 = mybir.dt.float32
BF16 = mybir.dt.bfloat16
AF = mybir.ActivationFunctionType
ALU = mybir.AluOpType
BF = ml_dtypes.bfloat16

B, N, C = 2, 2048, 768
H, HD = 12, 64
HP = 3            # heads per core
NCORES = 8
CCH = 6           # contraction chunks of 128
NT = 4            # token tiles of 512
KB = 16           # k blocks of 128
NG = 8            # 2-kb groups per (head, qtile) phase

SWAP_MASK = [(i + 16) % 32 for i in range(32)]
PERM = np.concatenate([np.arange(0, 16), np.arange(32, 48),
                       np.arange(16, 32), np.arange(48, 64)])
SIGN = np.where(PERM < 32, -1.0, 1.0).astype(np.float32)
# rope partner of PERM-position p (SWAP_MASK's intra-32 half swap)
SWAPIDX = np.array([(p // 32) * 32 + (p + 16) % 32 for p in range(64)])

_NC_CACHE = {}


def build_nc(split_waits=True):
    nc = bass.Bass(target_bir_lowering=True)
    xT = nc.declare_dram_parameter("xT", [C, N], BF16, isOutput=False)
    wqk = nc.declare_dram_parameter("wqk", [C, HP * 128], BF16, isOutput=False)
    cosw = nc.declare_dram_parameter("cosw", [128, N], BF16, isOutput=False)
    sinw = nc.declare_dram_parameter("sinw", [128, N], BF16, isOutput=False)
    wvp = nc.declare_dram_parameter("wvp", [128, CCH * HP * 64], BF16,
                                    isOutput=False)
    wp01 = nc.declare_dram_parameter("wp01", [128, C], BF16, isOutput=False)
    wp2 = nc.declare_dram_parameter("wp2", [64, C], BF16, isOutput=False)
    # consts: [onesp(2) | sel4(512) | ident(128)]
    consts = nc.declare_dram_parameter("consts", [128, 642], BF16,
                                       isOutput=False)
    bqk = nc.declare_dram_parameter("bqk", [128, HP], F32, isOutput=False)
    out = nc.declare_dram_parameter("out", [N, C], BF16, isOutput=True)

    with tile.TileContext(nc) as tc, ExitStack() as ctx:
        sb = ctx.enter_context(tc.tile_pool(name="sb", bufs=1))
        pipe = ctx.enter_context(tc.tile_pool(name="pipe", bufs=2))
        pxp = ctx.enter_context(tc.tile_pool(name="pxp", bufs=28))
        otp = ctx.enter_context(tc.tile_pool(name="otp", bufs=4))
        pop = ctx.enter_context(tc.tile_pool(name="pop", bufs=2))
        # PSUM: 4 + 2 + 1 + 1 = 8 banks
        sp = ctx.enter_context(tc.tile_pool(name="sp", bufs=2, space="PSUM"))
        qp = ctx.enter_context(tc.tile_pool(name="qp", bufs=2, space="PSUM"))
        op = ctx.enter_context(tc.tile_pool(name="op", bufs=1, space="PSUM"))
        mp = ctx.enter_context(tc.tile_pool(name="mp", bufs=1, space="PSUM"))

        # ---------- static SBUF tiles ----------
        xs = sb.tile([128, CCH, N], BF16, tag="xs")
        wqk_sb = sb.tile([128, CCH, HP * 128], BF16, tag="wqk")
        wv_sb = sb.tile([128, CCH, HP * 64], BF16, tag="wv")
        cos_sb = sb.tile([128, N], BF16, tag="cos")
        sin_sb = sb.tile([128, N], BF16, tag="sin")
        cn = sb.tile([128, 642], BF16, tag="cn")
        onesp_sb = cn[:, 0:2]
        sel_sb = cn[:, 2:514]
        ident_sb = cn[:, 514:642]
        bqk_sb = sb.tile([128, HP], F32, tag="bqk")
        wp01_sb = sb.tile([128, C], BF16, tag="wp01")
        wp2_sb = sb.tile([64, C], BF16, tag="wp2")

        q12 = sb.tile([128, N], BF16, tag="q12")
        k12 = sb.tile([128, N], BF16, tag="k12")
        q3 = sb.tile([64, N], BF16, tag="q3")
        k3 = sb.tile([64, N], BF16, tag="k3")
        t4_all = sb.tile([128, N], BF16, tag="t4_all")
        s_sb = sb.tile([128, 512], F32, tag="s_sb")
        sv = sb.tile([128, 512], BF16, tag="sv")
        v3 = sb.tile([128, KB, HP, 65], BF16, tag="v3")
        ones48 = sb.tile([128, KB * HP], BF16, tag="ones48")
        o2 = sb.tile([128, NT, 4, 128], BF16, tag="o2")
        o1 = sb.tile([128, NT, 4, 64], BF16, tag="o1")

        def qT(h):
            return (q12[0:64], q12[64:128], q3[:])[h]

        def kT(h):
            return (k12[0:64], k12[64:128], k3[:])[h]

        # ---------- prologue DMAs (ordered for earliest qkv start) ----------
        xT_r = xT[:].rearrange("(c p) n -> p c n", p=128)
        wqk_r = wqk[:].rearrange("(c p) m -> p c m", p=128)
        d = nc.sync.dma_start
        d(cn[:], consts[:, :])
        d(bqk_sb[:], bqk[:, :])
        d(wqk_sb[:, 0:2, :], wqk_r[:, 0:2, :])
        d(xs[:, 0:3, 0:512], xT_r[:, 0:3, 0:512])      # tile-0 tokens
        d(wqk_sb[:, 2:6, :], wqk_r[:, 2:6, :])
        d(xs[:, 3:6, 0:512], xT_r[:, 3:6, 0:512])
        d(cos_sb[:, 0:1024], cosw[:, 0:1024])
        d(sin_sb[:, 0:1024], sinw[:, 0:1024])
        d(xs[:, :, 512:1024], xT_r[:, :, 512:1024])
        d(xs[:, :, 1024:1536], xT_r[:, :, 1024:1536])
        d(wv_sb[:].rearrange("p c m -> p (c m)"), wvp[:, :])
        d(xs[:, :, 1536:2048], xT_r[:, :, 1536:2048])
        d(cos_sb[:, 1024:2048], cosw[:, 1024:2048])
        d(sin_sb[:, 1024:2048], sinw[:, 1024:2048])
        d(wp01_sb[:], wp01[:, :])
        d(wp2_sb[:], wp2[:, :])

        nc.vector.memset(sv[:], 1.0)   # rows never written stay 1 (sel zeros them)
        nc.vector.memset(s_sb[:], 1.0)
        nc.vector.memset(ones48[:], 1.0)
        nc.vector.tensor_copy(
            v3[:].rearrange("p a b n -> p (a b) n", n=65)[:, :, 64], ones48[:])

        def mm(out_ap, lhsT, rhs, start, stop):
            nc.tensor.matmul(out_ap, lhsT, rhs, start=start, stop=stop,
                             skip_group_check=True)

        # ---------- qkv-head generator ----------
        # Per-tile chunks: mms -> RoPE pipe -> finA (sumsq+rsqrt) ->
        # finB (broadcast+scale). Emission defers fins so PE stays dense;
        # the qp ring (2) tolerates exactly one deferred finA.
        def qkv_gen(h):
            hs = slice(h * 128, (h + 1) * 128)
            qk = [None] * NT

            def mms(t):
                ts = slice(t * 512, (t + 1) * 512)
                qk[t] = qp.tile([128, 512], F32, tag="q", name=f"qk{t}")
                for c in range(CCH):
                    mm(qk[t][:], wqk_sb[:, c, hs], xs[:, c, ts], c == 0,
                       c == CCH - 1)

            def rope(t):
                ts = slice(t * 512, (t + 1) * 512)
                qkb = pipe.tile([128, 512], BF16, tag="qkb")
                nc.vector.tensor_scalar(qkb[:], qk[t][:], bqk_sb[:, h:h + 1],
                                        None, ALU.add)
                sq = pipe.tile([128, 512], BF16, tag="sq")
                if h == 0:
                    nc.vector.tensor_mul(sq[:], qkb[:], qkb[:])
                t1 = pipe.tile([128, 512], BF16, tag="t1")
                nc.gpsimd.tensor_mul(t1[:], qkb[:], cos_sb[:, ts])
                t2 = pipe.tile([128, 512], BF16, tag="t2")
                nc.vector.stream_shuffle(t2[:], qkb[:], SWAP_MASK)
                t3 = pipe.tile([128, 512], BF16, tag="t3")
                nc.vector.tensor_mul(t3[:], t2[:], sin_sb[:, ts])
                nc.vector.tensor_add(t4_all[:, ts], t1[:], t3[:])
                if h != 0:
                    nc.gpsimd.tensor_mul(sq[:], qkb[:], qkb[:])
                return sq

            def finA(t, sq):
                rows = slice(32 * t, 32 * t + 2)
                mm(qk[t][0:2, :], onesp_sb[:], sq[:], True, True)
                if h == 0:
                    # rsqrt = exp(-0.5 ln(ms)); same ACT table as softmax exp
                    lv = pipe.tile([2, 512], F32, tag="lv", name="lv")
                    nc.scalar.activation(lv[:], qk[t][0:2, :], AF.Ln,
                                         bias=0.0, scale=1.0 / HD)
                    nc.scalar.activation(sv[rows, :], lv[:], AF.Exp,
                                         bias=0.0, scale=-0.5)
                else:
                    nc.vector.tensor_copy(s_sb[rows, :], qk[t][0:2, :])

            def lnexp():
                lva = pipe.tile([128, 512], F32, tag="lva", name="lva")
                nc.scalar.activation(lva[:], s_sb[:], AF.Ln,
                                     bias=0.0, scale=1.0 / HD)
                nc.scalar.activation(sv[:], lva[:], AF.Exp, bias=0.0,
                                     scale=-0.5)

            def finB(t):
                ts = slice(t * 512, (t + 1) * 512)
                sqk_ps = qp.tile([128, 512], F32, tag="q")
                mm(sqk_ps[:], sel_sb[:, t * 128:(t + 1) * 128], sv[:],
                   True, True)
                nc.vector.tensor_mul(qT(h)[:, ts], t4_all[0:64, ts],
                                     sqk_ps[0:64, :])
                nc.vector.tensor_mul(kT(h)[:, ts], t4_all[64:128, ts],
                                     sqk_ps[64:128, :])

            sqs = [None] * NT

            def do_mms(t):
                mms(t)
                sqs[t] = rope(t)

            do_mms(0)
            yield 4500
            do_mms(1)
            yield 4500
            finA(0, sqs[0])
            yield 700
            do_mms(2)
            yield 4500
            finA(1, sqs[1])
            if h == 0:
                finB(0)
            yield 1600
            finA(2, sqs[2])
            yield 700
            do_mms(3)
            yield 4500
            if h == 0:
                finB(1)
            yield 900
            finA(3, sqs[3])
            yield 700
            if h != 0:
                lnexp()
                yield 700
                finB(0)
                yield 900
                finB(1)
                yield 900
            finB(2)
            yield 900
            finB(3)
            yield 900

        # ---------- v generator ----------
        def v_gen():
            for tt in range(KB):
                v_ps = qp.tile([128, HP * 64], F32, tag="q")
                for c in range(CCH):
                    mm(v_ps[:], xs[:, c, tt * 128:(tt + 1) * 128],
                       wv_sb[:, c, :], c == 0, c == CCH - 1)
                nc.vector.tensor_copy(
                    v3[:, tt, :, 0:64],
                    v_ps[:, :].rearrange("p (h n) -> p h n", h=HP))
                yield 1500

        # ---------- proj of one (qtile, qblock) ----------
        mtr = [None]

        def proj_qb(qt, qb):
            if mtr[0] is None:
                mtr[0] = mp.tile([128, 4, 128], F32, tag="m", name="mtr")
            m = mtr[0]
            tr01 = m[:, qb, 0:64].bitcast(BF16)
            tr2 = m[0:64, qb, 64:128].bitcast(BF16)
            nc.tensor.transpose(tr01, o2[:, qt, qb, :], ident_sb[:])
            nc.tensor.transpose(tr2, o1[:, qt, qb, :], ident_sb[:])
            on_act = qt == 3   # ACT is idle once the last exps drain
            ot01 = otp.tile([128, 128], BF16, tag="ot01")
            ot2 = otp.tile([64, 128], BF16, tag="ot2")
            if on_act:
                nc.scalar.activation(ot01[:], tr01, AF.Copy, bias=0.0,
                                     scale=1.0)
                nc.vector.tensor_copy(ot2[:], tr2)
            else:
                nc.vector.tensor_copy(ot01[:], tr01)
                nc.vector.tensor_copy(ot2[:], tr2)
            po = pop.tile([128, C], BF16, tag="po")
            for half in range(2):
                cs = slice(half * 384, (half + 1) * 384)
                p_ps = qp.tile([128, 384], F32, tag="q")
                mm(p_ps[:], ot01[:], wp01_sb[:, cs], True, False)
                mm(p_ps[:], ot2[:], wp2_sb[:, cs], False, True)
                if on_act and half == 1:
                    nc.scalar.activation(po[:, cs], p_ps[:], AF.Copy,
                                         bias=0.0, scale=1.0)
                else:
                    nc.vector.tensor_copy(po[:, cs], p_ps[:])
            tb = qt * 4 + qb
            nc.sync.dma_start(out[tb * 128:(tb + 1) * 128, :], po[:])

        # ---------- filler pump ----------
        fillers = deque()
        debt = [0.0]

        def pump(budget):
            budget += debt[0]
            while budget > 0 and fillers:
                try:
                    budget -= next(fillers[0])
                except StopIteration:
                    fillers.popleft()
            debt[0] = min(budget, 3000.0)

        def ensure_done(gen):
            """Pump until `gen` has fully emitted (emission-order guard for
            cross-generator data deps)."""
            while gen in fillers:
                pump(100000)

        # ---------- attention stream ----------
        phases = [(h, qt) for h in range(HP) for qt in range(NT)]
        px_tiles = {}
        emitted = set()

        def emit_group(p, g):
            if (p, g) in emitted:
                return
            emitted.add((p, g))
            h, qt = phases[p]
            qs = slice(qt * 512, (qt + 1) * 512)
            s_ps = sp.tile([128, 1024], F32, tag="s")
            for j in range(2):
                kb = 2 * g + j
                mm(s_ps[:, j * 512:(j + 1) * 512],
                   kT(h)[:, kb * 128:(kb + 1) * 128], qT(h)[:, qs],
                   True, True)
            px = pxp.tile([128, 1024], BF16, tag="px")
            nc.scalar.activation(px[:], s_ps[:], AF.Exp, bias=0.0, scale=0.125)
            px_tiles[(p, g)] = px

        def close_gen(p):
            if p == 0:
                ensure_done(vg)   # PV reads v3; emission-order guard
            h, qt = phases[p]
            o_ps = op.tile([128, 4, 65], F32, tag="o")
            for qb in range(4):
                for g in range(NG):
                    px = px_tiles[(p, g)]
                    for j in range(2):
                        kb = 2 * g + j
                        mm(o_ps[:, qb, :],
                           px[:, j * 512 + qb * 128:j * 512 + (qb + 1) * 128],
                           v3[:, kb, h, :],
                           qb == 0 and kb == 0, kb == KB - 1)
                if qb == 1 or qb == 3:
                    yield
            # normalize by the ones-column denominators (batched reciprocal,
            # then per-qb per-partition multiply); epilogues after ALL PV so
            # coarse WAR tracking can't serialize the qb bundles
            rec4 = pipe.tile([128, 4], F32, tag="rec4", name="rec4")
            nc.vector.reciprocal(rec4[:], o_ps[:, :, 64])
            for qb in range(4):
                dst = (o2[:, qt, qb, h * 64:(h + 1) * 64] if h < 2
                       else o1[:, qt, qb, :])
                nc.vector.tensor_scalar(dst, o_ps[:, qb, 0:64],
                                        rec4[:, qb:qb + 1], None, ALU.mult)
                if h == 2:
                    proj_qb(qt, qb)
                yield
            for g in range(NG):
                del px_tiles[(p, g)]

        def drain_close(cg):
            for _ in cg:
                pass

        # ---------- main schedule ----------
        # Phase 0 runs with qkv(h0) inlined per tile: tile t unlocks S groups
        # 2t, 2t+1 (k-tiles) while qT(qt0) comes entirely from tile 0.
        qg0 = qkv_gen(0)
        vg = v_gen()
        g1, g2 = qkv_gen(1), qkv_gen(2)
        fillers.append(vg)
        fillers.append(g1)
        need_gen = {1: g1, 2: g2}

        def drain_n(gen, n_chunks):
            for _ in range(n_chunks):
                next(gen)

        # Front: dense qkv-h0 mms with the exp stream fed by EVERY group
        # whose gates are open. Tile t gates k-blocks 4t..4t+3 (groups
        # 2t,2t+1 of every h0 phase) and the q-tokens of phase (0,t).
        drain_n(qg0, 5)              # thru finB(0)
        front = [(0, 0), (0, 1),
                 "T1", (0, 2), (0, 3), (1, 0), (1, 1),
                 "T2", (0, 4), (0, 5), (1, 2), (1, 3), (2, 0), (2, 1),
                 "T3", (0, 6), (0, 7), (1, 4), (1, 5), (2, 2), (2, 3),
                 (3, 0), (3, 1)]
        for item in front:
            if item == "T1":
                drain_n(qg0, 3)      # finA2, mms3, finB1
            elif item == "T2":
                drain_n(qg0, 2)      # finA3, finB2
            elif item == "T3":
                drain_n(qg0, 1)      # finB3
            else:
                emit_group(*item)
                pump(600)

        # Steady state: early closes deferred ~2 phases (px ring holds ~3
        # phases) so v/qkv fillers use the early PE slack; late closes pulled
        # in so proj work overlaps the remaining exp stream.
        NP = len(phases)
        close_at = {}
        for p in range(NP - 1):
            # early closes deferred 3 phases (shifts PV out of the PE-heavy
            # qkv/v window); h2 closes pulled in so proj overlaps exps
            lag = (p + 3, 1) if p < 8 else (p + 1, 0)
            close_at.setdefault(lag, []).append(p)

        def after_close(cp):
            if phases[cp] == (0, 2):
                fillers.append(g2)

        active_closes = []
        for p in range(1, NP):
            nh = phases[p][0]
            if nh != phases[p - 1][0]:
                ensure_done(need_gen[nh])
            for g in range(NG):
                emit_group(p, g)
                pump(2400 if p <= 4 else 2200)
                for cp in close_at.get((p, g), []):
                    active_closes.append((cp, close_gen(cp)))
                if active_closes:
                    steps = 2 if p >= NP - 2 else 1
                    for _ in range(steps):
                        if not active_closes:
                            break
                        cp, cg = active_closes[0]
                        try:
                            next(cg)
                        except StopIteration:
                            active_closes.pop(0)
                            after_close(cp)
        for cp, cg in active_closes:
            drain_close(cg)
            after_close(cp)
        drain_close(close_gen(NP - 1))
        while fillers:
            pump(100000)

    if split_waits:
        _split_waits(nc)
    return nc


def _split_waits(nc):
    """Walrus lowers at most one sync-wait per instruction; move excess waits
    onto NoOps inserted just before, on the same engine queue."""
    k = 0
    for fn in nc.m.functions:
        for bb in fn.blocks:
            il = bb.instructions
            idx = 0
            while idx < len(il):
                inst = il[idx]
                si = inst.sync_info
                eng = getattr(inst, "engine", None)
                if (si is not None and len(si.on_wait) > 1
                        and eng is not None
                        and str(eng) != "EngineType.Unassigned"):
                    waits = list(si.on_wait)
                    inst.sync_info = mybir.SyncInfo(
                        on_wait=[waits[-1]], on_update=list(si.on_update))
                    for w in waits[:-1]:
                        nop = mybir.InstNoOp(
                            name=f"I-waitnop-{k}", engine=eng, ins=[], outs=[],
                            sync_info=mybir.SyncInfo(on_wait=[w], on_update=[]))
                        k += 1
                        il.insert(idx, nop)
                        idx += 1
                idx += 1


def _prep_core_inputs(core, x, rope_cos, rope_sin, qkv_kernel, qkv_bias,
                      proj_kernel, proj_bias, q_norm_w, k_norm_w):
    b = core // 4
    heads = [3 * (core % 4) + i for i in range(HP)]

    wq = qkv_kernel.reshape(C, 3, H, HD)
    bq = qkv_bias.reshape(3, H, HD)

    xTa = np.ascontiguousarray(x[b].T).astype(BF)

    wqk = np.empty((C, HP * 128), np.float32)
    bqk = np.zeros((128, HP), np.float32)
    for i, h in enumerate(heads):
        wqk[:, i * 128:i * 128 + 64] = wq[:, 0, h, PERM]
        wqk[:, i * 128 + 64:(i + 1) * 128] = wq[:, 1, h, PERM]
        bqk[0:64, i] = bq[0, h, PERM]
        bqk[64:128, i] = bq[1, h, PERM]

    wv = np.zeros((C, HP * 64), np.float32)
    for i, h in enumerate(heads):
        wv[:, i * 64:(i + 1) * 64] = wq[:, 2, h, :]
    # packed [p, (c m)] so the SBUF copy is one dense DMA
    wvp = wv.reshape(CCH, 128, HP * 64).transpose(1, 0, 2).reshape(128, -1)

    cosT = rope_cos.T  # (HD, N)
    sinT = rope_sin.T
    cosw = np.empty((128, N), np.float32)
    sinw = np.empty((128, N), np.float32)
    cosw[0:64] = cosT[PERM] * q_norm_w[PERM][:, None]
    cosw[64:128] = cosT[PERM] * k_norm_w[PERM][:, None]
    # sin multiplies the SHUFFLED (partner) value -> partner's norm weight
    qn_p = q_norm_w[PERM][SWAPIDX]
    kn_p = k_norm_w[PERM][SWAPIDX]
    sinw[0:64] = SIGN[:, None] * sinT[PERM] * qn_p[:, None]
    sinw[64:128] = SIGN[:, None] * sinT[PERM] * kn_p[:, None]

    onesp = np.zeros((128, 2), np.float32)
    onesp[0:64, 0] = 1.0
    onesp[64:128, 1] = 1.0

    sel4 = np.zeros((128, 512), np.float32)
    for t in range(NT):
        sel4[32 * t, t * 128:t * 128 + 64] = 1.0
        sel4[32 * t + 1, t * 128 + 64:(t + 1) * 128] = 1.0

    rows01 = np.concatenate([np.arange(h * HD, (h + 1) * HD)
                             for h in heads[0:2]])
    rows2 = np.arange(heads[2] * HD, (heads[2] + 1) * HD)
    wp01 = proj_kernel[rows01, :]
    wp2 = proj_kernel[rows2, :]

    consts = np.zeros((128, 642), np.float32)
    consts[:, 0:2] = onesp
    consts[:, 2:514] = sel4
    consts[:, 514:642] = np.eye(128, dtype=np.float32)
    return {"xT": xTa, "wqk": wqk.astype(BF), "bqk": bqk,
            "cosw": cosw.astype(BF), "sinw": sinw.astype(BF),
            "wvp": np.ascontiguousarray(wvp).astype(BF),
            "wp01": np.ascontiguousarray(wp01).astype(BF),
            "wp2": np.ascontiguousarray(wp2).astype(BF),
            "consts": consts.astype(BF)}


def kernel(x, rope_cos, rope_sin, qkv_kernel, qkv_bias, proj_kernel,
           proj_bias, q_norm_w, k_norm_w, _trace=False):
    args = [np.asarray(a, dtype=np.float32) for a in
            (x, rope_cos, rope_sin, qkv_kernel, qkv_bias, proj_kernel,
             proj_bias, q_norm_w, k_norm_w)]
    in_maps = [_prep_core_inputs(c, *args) for c in range(NCORES)]

    if "nc" not in _NC_CACHE:
        _NC_CACHE["nc"] = build_nc()
    nc = _NC_CACHE["nc"]

    res = run_bass_kernel_spmd(nc, in_maps, core_ids=list(range(NCORES)),
                               trace=_trace)
    parts = [np.asarray(res.results[c]["out"]).astype(np.float32)
             for c in range(NCORES)]
    # v-bias contributes exactly bv @ proj_kernel (softmax rows sum to 1)
    pb = (np.asarray(proj_bias, dtype=np.float32)
          + np.asarray(qkv_bias, dtype=np.float32)[2 * C:]
          @ np.asarray(proj_kernel, dtype=np.float32))
    out = np.empty((B, N, C), np.float32)
    for b in range(B):
        out[b] = (parts[4 * b] + parts[4 * b + 1] + parts[4 * b + 2]
                  + parts[4 * b + 3] + pb)
    if _trace:
        kernel.last_results = res
    return out


# revision 6
# speedup vs baseline: 1.6057x; 1.0025x over previous
"""Multi-head attention (RMSNorm-QK + RoPE + softmax + proj) on 8 Trainium2 cores.

v2 design (cost-model-driven rewrite of the baseline):
 - bf16 operands everywhere (matmuls cost 1 cyc/row like fp32r, but DVE gets
   2x modes and DMA halves); fp32 PSUM accumulation throughout.
 - Transposed PV: O tiles are [128 q, 65] (64 dims + ones col for the softmax
   denominator), using all 128 output partitions -> PV drops from 32768 to
   16640 cyc/head, the denominator becomes a per-partition column (one DVE
   tensor_scalar divide), and the old broadcast-reciprocal matmuls vanish.
 - O^T for the projection comes from PE transposes (128 bf16 rows each).
 - RMS rsqrt on DVE ((x/64)^-0.5 via tensor_scalar pow), qkv bias added in the
   DVE pipeline (per-partition scalar), v bias folded into the host-side proj
   bias (softmax rows sum to 1), so ACT runs the softmax exp ONLY.
 - RoPE elementwise work split DVE/Pool; emission order software-pipelines
   S(k+1) ahead of exp(k), stages a phase's px tiles in SBUF so each O
   qb-region accumulates contiguously (PSUM start bit stays per-element
   correct on HW), defers phase closes ~1.25 phases so early PE work (qkv+v)
   overlaps the ACT-bound exp stream, and pumps qkv/v/proj filler chunks into
   the PE gaps.

Sharding: core c handles batch c//4 and heads [3*(c%4), 3*(c%4)+3).
Each core writes a bf16 [N, C] partial; the host sums 4 partials per batch
and adds proj_bias + qkv_bias[v-part] @ proj_kernel.
"""
import sys

for _p in ("/opt/trn_rl_repo", "/opt/trn_rl_repo/concourse"):
    if _p not in sys.path:
        sys.path.insert(0, _p)

from collections import deque
from contextlib import ExitStack

import ml_dtypes
import numpy as np

import concourse.bass as bass
import concourse.mybir as mybir
import concourse.tile as tile
from concourse.bass_utils import run_bass_kernel_spmd

F32 = mybir.dt.float32
BF16 = mybir.dt.bfloat16
AF = mybir.ActivationFunctionType
ALU = mybir.AluOpType
BF = ml_dtypes.bfloat16

B, N, C = 2, 2048, 768
H, HD = 12, 64
HP = 3            # heads per core
NCORES = 8
CCH = 6           # contraction chunks of 128
NT = 4            # token tiles of 512
KB = 16           # k blocks of 128
NG = 8            # 2-kb groups per (head, qtile) phase

SWAP_MASK = [(i + 16) % 32 for i in range(32)]
PERM = np.concatenate([np.arange(0, 16), np.arange(32, 48),
                       np.arange(16, 32), np.arange(48, 64)])
SIGN = np.where(PERM < 32, -1.0, 1.0).astype(np.float32)
# rope partner of PERM-position p (SWAP_MASK's intra-32 half swap)
SWAPIDX = np.array([(p // 32) * 32 + (p + 16) % 32 for p in range(64)])

_NC_CACHE = {}


def build_nc(split_waits=True):
    nc = bass.Bass(target_bir_lowering=True)
    xT = nc.declare_dram_parameter("xT", [C, N], BF16, isOutput=False)
    wqk = nc.declare_dram_parameter("wqk", [C, HP * 128], BF16, isOutput=False)
    cosw = nc.declare_dram_parameter("cosw", [128, N], BF16, isOutput=False)
    sinw = nc.declare_dram_parameter("sinw", [128, N], BF16, isOutput=False)
    wvp = nc.declare_dram_parameter("wvp", [128, CCH * HP * 64], BF16,
                                    isOutput=False)
    wp01 = nc.declare_dram_parameter("wp01", [128, C], BF16, isOutput=False)
    wp2 = nc.declare_dram_parameter("wp2", [64, C], BF16, isOutput=False)
    # consts: [onesp(2) | sel4(512) | ident(128)]
    consts = nc.declare_dram_parameter("consts", [128, 642], BF16,
                                       isOutput=False)
    bqk = nc.declare_dram_parameter("bqk", [128, HP], F32, isOutput=False)
    out = nc.declare_dram_parameter("out", [N, C], BF16, isOutput=True)

    with tile.TileContext(nc) as tc, ExitStack() as ctx:
        sb = ctx.enter_context(tc.tile_pool(name="sb", bufs=1))
        pipe = ctx.enter_context(tc.tile_pool(name="pipe", bufs=2))
        pxp = ctx.enter_context(tc.tile_pool(name="pxp", bufs=28))
        otp = ctx.enter_context(tc.tile_pool(name="otp", bufs=6))
        pop = ctx.enter_context(tc.tile_pool(name="pop", bufs=3))
        # PSUM: 4 + 2 + 1 + 1 = 8 banks
        sp = ctx.enter_context(tc.tile_pool(name="sp", bufs=2, space="PSUM"))
        qp = ctx.enter_context(tc.tile_pool(name="qp", bufs=2, space="PSUM"))
        op = ctx.enter_context(tc.tile_pool(name="op", bufs=1, space="PSUM"))
        mp = ctx.enter_context(tc.tile_pool(name="mp", bufs=1, space="PSUM"))

        # ---------- static SBUF tiles ----------
        xs = sb.tile([128, CCH, N], BF16, tag="xs")
        wqk_sb = sb.tile([128, CCH, HP * 128], BF16, tag="wqk")
        wv_sb = sb.tile([128, CCH, HP * 64], BF16, tag="wv")
        cos_sb = sb.tile([128, N], BF16, tag="cos")
        sin_sb = sb.tile([128, N], BF16, tag="sin")
        cn = sb.tile([128, 642], BF16, tag="cn")
        onesp_sb = cn[:, 0:2]
        sel_sb = cn[:, 2:514]
        ident_sb = cn[:, 514:642]
        bqk_sb = sb.tile([128, HP], F32, tag="bqk")
        wp01_sb = sb.tile([128, C], BF16, tag="wp01")
        wp2_sb = sb.tile([64, C], BF16, tag="wp2")

        q12 = sb.tile([128, N], BF16, tag="q12")
        k12 = sb.tile([128, N], BF16, tag="k12")
        q3 = sb.tile([64, N], BF16, tag="q3")
        k3 = sb.tile([64, N], BF16, tag="k3")
        t4_all = sb.tile([128, N], BF16, tag="t4_all")
        s_sb = sb.tile([128, 512], F32, tag="s_sb")
        sv = sb.tile([128, 512], BF16, tag="sv")
        v3 = sb.tile([128, KB, HP, 65], BF16, tag="v3")
        ones48 = sb.tile([128, KB * HP], BF16, tag="ones48")
        o2 = sb.tile([128, NT, 4, 128], BF16, tag="o2")
        o1 = sb.tile([128, NT, 4, 64], BF16, tag="o1")

        def qT(h):
            return (q12[0:64], q12[64:128], q3[:])[h]

        def kT(h):
            return (k12[0:64], k12[64:128], k3[:])[h]

        # ---------- prologue DMAs (ordered for earliest qkv start) ----------
        xT_r = xT[:].rearrange("(c p) n -> p c n", p=128)
        wqk_r = wqk[:].rearrange("(c p) m -> p c m", p=128)
        d = nc.sync.dma_start
        d(cn[:], consts[:, :])
        d(bqk_sb[:], bqk[:, :])
        d(wqk_sb[:, 0:2, :], wqk_r[:, 0:2, :])
        d(xs[:, 0:3, 0:512], xT_r[:, 0:3, 0:512])      # tile-0 tokens
        d(wqk_sb[:, 2:6, :], wqk_r[:, 2:6, :])
        d(xs[:, 3:6, 0:512], xT_r[:, 3:6, 0:512])
        d(cos_sb[:, 0:1024], cosw[:, 0:1024])
        d(sin_sb[:, 0:1024], sinw[:, 0:1024])
        d(xs[:, :, 512:1024], xT_r[:, :, 512:1024])
        d(xs[:, :, 1024:1536], xT_r[:, :, 1024:1536])
        d(wv_sb[:].rearrange("p c m -> p (c m)"), wvp[:, :])
        d(xs[:, :, 1536:2048], xT_r[:, :, 1536:2048])
        d(cos_sb[:, 1024:2048], cosw[:, 1024:2048])
        d(sin_sb[:, 1024:2048], sinw[:, 1024:2048])
        d(wp01_sb[:], wp01[:, :])
        d(wp2_sb[:], wp2[:, :])

        nc.vector.memset(sv[:], 1.0)   # rows never written stay 1 (sel zeros them)
        nc.vector.memset(s_sb[:], 1.0)
        nc.vector.memset(ones48[:], 1.0)
        nc.vector.tensor_copy(
            v3[:].rearrange("p a b n -> p (a b) n", n=65)[:, :, 64], ones48[:])

        def mm(out_ap, lhsT, rhs, start, stop):
            nc.tensor.matmul(out_ap, lhsT, rhs, start=start, stop=stop,
                             skip_group_check=True)

        # ---------- qkv-head generator ----------
        # Per-tile chunks: mms -> RoPE pipe -> finA (sumsq+rsqrt) ->
        # finB (broadcast+scale). Emission defers fins so PE stays dense;
        # the qp ring (2) tolerates exactly one deferred finA.
        def qkv_gen(h):
            hs = slice(h * 128, (h + 1) * 128)
            qk = [None] * NT

            def mms(t):
                ts = slice(t * 512, (t + 1) * 512)
                qk[t] = qp.tile([128, 512], F32, tag="q", name=f"qk{t}")
                for c in range(CCH):
                    mm(qk[t][:], wqk_sb[:, c, hs], xs[:, c, ts], c == 0,
                       c == CCH - 1)

            def rope(t):
                ts = slice(t * 512, (t + 1) * 512)
                qkb = pipe.tile([128, 512], BF16, tag="qkb")
                nc.vector.tensor_scalar(qkb[:], qk[t][:], bqk_sb[:, h:h + 1],
                                        None, ALU.add)
                sq = pipe.tile([128, 512], BF16, tag="sq")
                if h == 0:
                    nc.vector.tensor_mul(sq[:], qkb[:], qkb[:])
                t1 = pipe.tile([128, 512], BF16, tag="t1")
                nc.gpsimd.tensor_mul(t1[:], qkb[:], cos_sb[:, ts])
                t2 = pipe.tile([128, 512], BF16, tag="t2")
                nc.vector.stream_shuffle(t2[:], qkb[:], SWAP_MASK)
                t3 = pipe.tile([128, 512], BF16, tag="t3")
                nc.vector.tensor_mul(t3[:], t2[:], sin_sb[:, ts])
                nc.vector.tensor_add(t4_all[:, ts], t1[:], t3[:])
                if h != 0:
                    nc.gpsimd.tensor_mul(sq[:], qkb[:], qkb[:])
                return sq

            def finA(t, sq):
                rows = slice(32 * t, 32 * t + 2)
                mm(qk[t][0:2, :], onesp_sb[:], sq[:], True, True)
                if h == 0:
                    # rsqrt = exp(-0.5 ln(ms)); same ACT table as softmax exp
                    lv = pipe.tile([2, 512], F32, tag="lv", name="lv")
                    nc.scalar.activation(lv[:], qk[t][0:2, :], AF.Ln,
                                         bias=0.0, scale=1.0 / HD)
                    nc.scalar.activation(sv[rows, :], lv[:], AF.Exp,
                                         bias=0.0, scale=-0.5)
                else:
                    nc.vector.tensor_copy(s_sb[rows, :], qk[t][0:2, :])

            def lnexp():
                lva = pipe.tile([128, 512], F32, tag="lva", name="lva")
                nc.scalar.activation(lva[:], s_sb[:], AF.Ln,
                                     bias=0.0, scale=1.0 / HD)
                nc.scalar.activation(sv[:], lva[:], AF.Exp, bias=0.0,
                                     scale=-0.5)

            def finB(t):
                ts = slice(t * 512, (t + 1) * 512)
                sqk_ps = qp.tile([128, 512], F32, tag="q")
                mm(sqk_ps[:], sel_sb[:, t * 128:(t + 1) * 128], sv[:],
                   True, True)
                nc.vector.tensor_mul(qT(h)[:, ts], t4_all[0:64, ts],
                                     sqk_ps[0:64, :])
                nc.vector.tensor_mul(kT(h)[:, ts], t4_all[64:128, ts],
                                     sqk_ps[64:128, :])

            sqs = [None] * NT

            def do_mms(t):
                mms(t)
                sqs[t] = rope(t)

            do_mms(0)
            yield 4500
            do_mms(1)
            yield 4500
            finA(0, sqs[0])
            yield 700
            do_mms(2)
            yield 4500
            finA(1, sqs[1])
            if h == 0:
                finB(0)
            yield 1600
            finA(2, sqs[2])
            yield 700
            do_mms(3)
            yield 4500
            if h == 0:
                finB(1)
            yield 900
            finA(3, sqs[3])
            yield 700
            if h != 0:
                lnexp()
                yield 700
                finB(0)
                yield 900
                finB(1)
                yield 900
            finB(2)
            yield 900
            finB(3)
            yield 900

        # ---------- v generator ----------
        def v_gen():
            for tt in range(KB):
                v_ps = qp.tile([128, HP * 64], F32, tag="q")
                for c in range(CCH):
                    mm(v_ps[:], xs[:, c, tt * 128:(tt + 1) * 128],
                       wv_sb[:, c, :], c == 0, c == CCH - 1)
                nc.vector.tensor_copy(
                    v3[:, tt, :, 0:64],
                    v_ps[:, :].rearrange("p (h n) -> p h n", h=HP))
                yield 1500

        # ---------- proj of one (qtile, qblock) ----------
        mtr = [None]

        def proj_qb(qt, qb):
            if mtr[0] is None:
                mtr[0] = mp.tile([128, 4, 128], F32, tag="m", name="mtr")
            m = mtr[0]
            tr01 = m[:, qb, 0:64].bitcast(BF16)
            tr2 = m[0:64, qb, 64:128].bitcast(BF16)
            nc.tensor.transpose(tr01, o2[:, qt, qb, :], ident_sb[:])
            nc.tensor.transpose(tr2, o1[:, qt, qb, :], ident_sb[:])
            on_act = qt == 3   # ACT is idle once the last exps drain
            ot01 = otp.tile([128, 128], BF16, tag="ot01")
            ot2 = otp.tile([64, 128], BF16, tag="ot2")
            if on_act:
                nc.scalar.activation(ot01[:], tr01, AF.Copy, bias=0.0,
                                     scale=1.0)
                nc.vector.tensor_copy(ot2[:], tr2)
            else:
                nc.vector.tensor_copy(ot01[:], tr01)
                nc.vector.tensor_copy(ot2[:], tr2)
            po = pop.tile([128, C], BF16, tag="po")
            for half in range(2):
                cs = slice(half * 384, (half + 1) * 384)
                p_ps = qp.tile([128, 384], F32, tag="q")
                mm(p_ps[:], ot01[:], wp01_sb[:, cs], True, False)
                mm(p_ps[:], ot2[:], wp2_sb[:, cs], False, True)
                if on_act and half == 1:
                    nc.scalar.activation(po[:, cs], p_ps[:], AF.Copy,
                                         bias=0.0, scale=1.0)
                else:
                    nc.vector.tensor_copy(po[:, cs], p_ps[:])
            tb = qt * 4 + qb
            nc.sync.dma_start(out[tb * 128:(tb + 1) * 128, :], po[:])

        # ---------- filler pump ----------
        fillers = deque()
        debt = [0.0]

        def pump(budget):
            budget += debt[0]
            while budget > 0 and fillers:
                try:
                    budget -= next(fillers[0])
                except StopIteration:
                    fillers.popleft()
            debt[0] = min(budget, 3000.0)

        def ensure_done(gen):
            """Pump until `gen` has fully emitted (emission-order guard for
            cross-generator data deps)."""
            while gen in fillers:
                pump(100000)

        # ---------- attention stream ----------
        phases = [(h, qt) for h in range(HP) for qt in range(NT)]
        px_tiles = {}
        emitted = set()

        def emit_group(p, g):
            if (p, g) in emitted:
                return
            emitted.add((p, g))
            h, qt = phases[p]
            qs = slice(qt * 512, (qt + 1) * 512)
            s_ps = sp.tile([128, 1024], F32, tag="s")
            for j in range(2):
                kb = 2 * g + j
                mm(s_ps[:, j * 512:(j + 1) * 512],
                   kT(h)[:, kb * 128:(kb + 1) * 128], qT(h)[:, qs],
                   True, True)
            px = pxp.tile([128, 1024], BF16, tag="px")
            nc.scalar.activation(px[:], s_ps[:], AF.Exp, bias=0.0, scale=0.125)
            px_tiles[(p, g)] = px

        def close_gen(p):
            if p == 0:
                ensure_done(vg)   # PV reads v3; emission-order guard
            h, qt = phases[p]
            o_ps = op.tile([128, 4, 65], F32, tag="o")
            for qb in range(4):
                for g in range(NG):
                    px = px_tiles[(p, g)]
                    for j in range(2):
                        kb = 2 * g + j
                        mm(o_ps[:, qb, :],
                           px[:, j * 512 + qb * 128:j * 512 + (qb + 1) * 128],
                           v3[:, kb, h, :],
                           qb == 0 and kb == 0, kb == KB - 1)
                if qb == 1 or qb == 3:
                    yield
            # normalize by the ones-column denominators (batched reciprocal,
            # then per-qb per-partition multiply); epilogues after ALL PV so
            # coarse WAR tracking can't serialize the qb bundles
            rec4 = pipe.tile([128, 4], F32, tag="rec4", name="rec4")
            nc.vector.reciprocal(rec4[:], o_ps[:, :, 64])
            for qb in range(4):
                dst = (o2[:, qt, qb, h * 64:(h + 1) * 64] if h < 2
                       else o1[:, qt, qb, :])
                nc.vector.tensor_scalar(dst, o_ps[:, qb, 0:64],
                                        rec4[:, qb:qb + 1], None, ALU.mult)
                if h == 2:
                    proj_qb(qt, qb)
                yield
            for g in range(NG):
                del px_tiles[(p, g)]

        def drain_close(cg):
            for _ in cg:
                pass

        # ---------- main schedule ----------
        # Phase 0 runs with qkv(h0) inlined per tile: tile t unlocks S groups
        # 2t, 2t+1 (k-tiles) while qT(qt0) comes entirely from tile 0.
        qg0 = qkv_gen(0)
        vg = v_gen()
        g1, g2 = qkv_gen(1), qkv_gen(2)
        fillers.append(vg)
        fillers.append(g1)
        need_gen = {1: g1, 2: g2}

        def drain_n(gen, n_chunks):
            for _ in range(n_chunks):
                next(gen)

        # Front: dense qkv-h0 mms with the exp stream fed by EVERY group
        # whose gates are open. Tile t gates k-blocks 4t..4t+3 (groups
        # 2t,2t+1 of every h0 phase) and the q-tokens of phase (0,t).
        drain_n(qg0, 5)              # thru finB(0)
        front = [(0, 0), (0, 1),
                 "T1", (0, 2), (0, 3), (1, 0), (1, 1),
                 "T2", (0, 4), (0, 5), (1, 2), (1, 3), (2, 0), (2, 1),
                 "T3", (0, 6), (0, 7), (1, 4), (1, 5), (2, 2), (2, 3),
                 (3, 0), (3, 1)]
        for item in front:
            if item == "T1":
                drain_n(qg0, 3)      # finA2, mms3, finB1
            elif item == "T2":
                drain_n(qg0, 2)      # finA3, finB2
            elif item == "T3":
                drain_n(qg0, 1)      # finB3
            else:
                emit_group(*item)
                pump(600)

        # Steady state: early closes deferred ~2 phases (px ring holds ~3
        # phases) so v/qkv fillers use the early PE slack; late closes pulled
        # in so proj work overlaps the remaining exp stream.
        NP = len(phases)
        close_at = {}
        for p in range(NP - 1):
            # early closes deferred 3 phases (shifts PV out of the PE-heavy
            # qkv/v window); h2 closes pulled in so proj overlaps exps
            lag = (p + 3, 1) if p < 8 else (p + 1, 0)
            close_at.setdefault(lag, []).append(p)

        def after_close(cp):
            if phases[cp] == (0, 2):
                fillers.append(g2)

        active_closes = []
        for p in range(1, NP):
            nh = phases[p][0]
            if nh != phases[p - 1][0]:
                ensure_done(need_gen[nh])
            for g in range(NG):
                emit_group(p, g)
                pump(2400 if p <= 4 else 2200)
                for cp in close_at.get((p, g), []):
                    active_closes.append((cp, close_gen(cp)))
                if active_closes:
                    steps = 2 if p >= NP - 2 else 1
                    for _ in range(steps):
                        if not active_closes:
                            break
                        cp, cg = active_closes[0]
                        try:
                            next(cg)
                        except StopIteration:
                            active_closes.pop(0)
                            after_close(cp)
        for cp, cg in active_closes:
            drain_close(cg)
            after_close(cp)
        drain_close(close_gen(NP - 1))
        while fillers:
            pump(100000)

    if split_waits:
        _split_waits(nc)
    return nc


def _split_waits(nc):
    """Walrus lowers at most one sync-wait per instruction; move excess waits
    onto NoOps inserted just before, on the same engine queue."""
    k = 0
    for fn in nc.m.functions:
        for bb in fn.blocks:
            il = bb.instructions
            idx = 0
            while idx < len(il):
                inst = il[idx]
                si = inst.sync_info
                eng = getattr(inst, "engine", None)
                if (si is not None and len(si.on_wait) > 1
                        and eng is not None
                        and str(eng) != "EngineType.Unassigned"):
                    waits = list(si.on_wait)
                    inst.sync_info = mybir.SyncInfo(
                        on_wait=[waits[-1]], on_update=list(si.on_update))
                    for w in waits[:-1]:
                        nop = mybir.InstNoOp(
                            name=f"I-waitnop-{k}", engine=eng, ins=[], outs=[],
                            sync_info=mybir.SyncInfo(on_wait=[w], on_update=[]))
                        k += 1
                        il.insert(idx, nop)
                        idx += 1
                idx += 1


def _prep_core_inputs(core, x, rope_cos, rope_sin, qkv_kernel, qkv_bias,
                      proj_kernel, proj_bias, q_norm_w, k_norm_w):
    b = core // 4
    heads = [3 * (core % 4) + i for i in range(HP)]

    wq = qkv_kernel.reshape(C, 3, H, HD)
    bq = qkv_bias.reshape(3, H, HD)

    xTa = np.ascontiguousarray(x[b].T).astype(BF)

    wqk = np.empty((C, HP * 128), np.float32)
    bqk = np.zeros((128, HP), np.float32)
    for i, h in enumerate(heads):
        wqk[:, i * 128:i * 128 + 64] = wq[:, 0, h, PERM]
        wqk[:, i * 128 + 64:(i + 1) * 128] = wq[:, 1, h, PERM]
        bqk[0:64, i] = bq[0, h, PERM]
        bqk[64:128, i] = bq[1, h, PERM]

    wv = np.zeros((C, HP * 64), np.float32)
    for i, h in enumerate(heads):
        wv[:, i * 64:(i + 1) * 64] = wq[:, 2, h, :]
    # packed [p, (c m)] so the SBUF copy is one dense DMA
    wvp = wv.reshape(CCH, 128, HP * 64).transpose(1, 0, 2).reshape(128, -1)

    cosT = rope_cos.T  # (HD, N)
    sinT = rope_sin.T
    cosw = np.empty((128, N), np.float32)
    sinw = np.empty((128, N), np.float32)
    cosw[0:64] = cosT[PERM] * q_norm_w[PERM][:, None]
    cosw[64:128] = cosT[PERM] * k_norm_w[PERM][:, None]
    # sin multiplies the SHUFFLED (partner) value -> partner's norm weight
    qn_p = q_norm_w[PERM][SWAPIDX]
    kn_p = k_norm_w[PERM][SWAPIDX]
    sinw[0:64] = SIGN[:, None] * sinT[PERM] * qn_p[:, None]
    sinw[64:128] = SIGN[:, None] * sinT[PERM] * kn_p[:, None]

    onesp = np.zeros((128, 2), np.float32)
    onesp[0:64, 0] = 1.0
    onesp[64:128, 1] = 1.0

    sel4 = np.zeros((128, 512), np.float32)
    for t in range(NT):
        sel4[32 * t, t * 128:t * 128 + 64] = 1.0
        sel4[32 * t + 1, t * 128 + 64:(t + 1) * 128] = 1.0

    rows01 = np.concatenate([np.arange(h * HD, (h + 1) * HD)
                             for h in heads[0:2]])
    rows2 = np.arange(heads[2] * HD, (heads[2] + 1) * HD)
    wp01 = proj_kernel[rows01, :]
    wp2 = proj_kernel[rows2, :]

    consts = np.zeros((128, 642), np.float32)
    consts[:, 0:2] = onesp
    consts[:, 2:514] = sel4
    consts[:, 514:642] = np.eye(128, dtype=np.float32)
    return {"xT": xTa, "wqk": wqk.astype(BF), "bqk": bqk,
            "cosw": cosw.astype(BF), "sinw": sinw.astype(BF),
            "wvp": np.ascontiguousarray(wvp).astype(BF),
            "wp01": np.ascontiguousarray(wp01).astype(BF),
            "wp2": np.ascontiguousarray(wp2).astype(BF),
            "consts": consts.astype(BF)}


def kernel(x, rope_cos, rope_sin, qkv_kernel, qkv_bias, proj_kernel,
           proj_bias, q_norm_w, k_norm_w, _trace=False):
    args = [np.asarray(a, dtype=np.float32) for a in
            (x, rope_cos, rope_sin, qkv_kernel, qkv_bias, proj_kernel,
             proj_bias, q_norm_w, k_norm_w)]
    in_maps = [_prep_core_inputs(c, *args) for c in range(NCORES)]

    if "nc" not in _NC_CACHE:
        _NC_CACHE["nc"] = build_nc()
    nc = _NC_CACHE["nc"]

    res = run_bass_kernel_spmd(nc, in_maps, core_ids=list(range(NCORES)),
                               trace=_trace)
    parts = [np.asarray(res.results[c]["out"]).astype(np.float32)
             for c in range(NCORES)]
    # v-bias contributes exactly bv @ proj_kernel (softmax rows sum to 1)
    pb = (np.asarray(proj_bias, dtype=np.float32)
          + np.asarray(qkv_bias, dtype=np.float32)[2 * C:]
          @ np.asarray(proj_kernel, dtype=np.float32))
    out = np.empty((B, N, C), np.float32)
    for b in range(B):
        out[b] = (parts[4 * b] + parts[4 * b + 1] + parts[4 * b + 2]
                  + parts[4 * b + 3] + pb)
    if _trace:
        kernel.last_results = res
    return out


# revision 7
# speedup vs baseline: 1.6140x; 1.0052x over previous
"""Multi-head attention (RMSNorm-QK + RoPE + softmax + proj) on 8 Trainium2 cores.

v2 design (cost-model-driven rewrite of the baseline):
 - bf16 operands everywhere (matmuls cost 1 cyc/row like fp32r, but DVE gets
   2x modes and DMA halves); fp32 PSUM accumulation throughout.
 - Transposed PV: O tiles are [128 q, 65] (64 dims + ones col for the softmax
   denominator), using all 128 output partitions -> PV drops from 32768 to
   16640 cyc/head, the denominator becomes a per-partition column (one DVE
   tensor_scalar divide), and the old broadcast-reciprocal matmuls vanish.
 - O^T for the projection comes from PE transposes (128 bf16 rows each).
 - RMS rsqrt on DVE ((x/64)^-0.5 via tensor_scalar pow), qkv bias added in the
   DVE pipeline (per-partition scalar), v bias folded into the host-side proj
   bias (softmax rows sum to 1), so ACT runs the softmax exp ONLY.
 - RoPE elementwise work split DVE/Pool; emission order software-pipelines
   S(k+1) ahead of exp(k), stages a phase's px tiles in SBUF so each O
   qb-region accumulates contiguously (PSUM start bit stays per-element
   correct on HW), defers phase closes ~1.25 phases so early PE work (qkv+v)
   overlaps the ACT-bound exp stream, and pumps qkv/v/proj filler chunks into
   the PE gaps.

Sharding: core c handles batch c//4 and heads [3*(c%4), 3*(c%4)+3).
Each core writes a bf16 [N, C] partial; the host sums 4 partials per batch
and adds proj_bias + qkv_bias[v-part] @ proj_kernel.
"""
import sys

for _p in ("/opt/trn_rl_repo", "/opt/trn_rl_repo/concourse"):
    if _p not in sys.path:
        sys.path.insert(0, _p)

from collections import deque
from contextlib import ExitStack

import ml_dtypes
import numpy as np

import concourse.bass as bass
import concourse.mybir as mybir
import concourse.tile as tile
from concourse.bass_utils import run_bass_kernel_spmd

F32 = mybir.dt.float32
BF16 = mybir.dt.bfloat16
AF = mybir.ActivationFunctionType
ALU = mybir.AluOpType
BF = ml_dtypes.bfloat16

B, N, C = 2, 2048, 768
H, HD = 12, 64
HP = 3            # heads per core
NCORES = 8
CCH = 6           # contraction chunks of 128
NT = 4            # token tiles of 512
KB = 16           # k blocks of 128
NG = 8            # 2-kb groups per (head, qtile) phase

SWAP_MASK = [(i + 16) % 32 for i in range(32)]
PERM = np.concatenate([np.arange(0, 16), np.arange(32, 48),
                       np.arange(16, 32), np.arange(48, 64)])
SIGN = np.where(PERM < 32, -1.0, 1.0).astype(np.float32)
# rope partner of PERM-position p (SWAP_MASK's intra-32 half swap)
SWAPIDX = np.array([(p // 32) * 32 + (p + 16) % 32 for p in range(64)])

_NC_CACHE = {}


def build_nc(split_waits=True):
    nc = bass.Bass(target_bir_lowering=True)
    xT = nc.declare_dram_parameter("xT", [C, N], BF16, isOutput=False)
    wqk = nc.declare_dram_parameter("wqk", [C, HP * 128], BF16, isOutput=False)
    cosw = nc.declare_dram_parameter("cosw", [128, N], BF16, isOutput=False)
    sinw = nc.declare_dram_parameter("sinw", [128, N], BF16, isOutput=False)
    wvp = nc.declare_dram_parameter("wvp", [128, CCH * HP * 64], BF16,
                                    isOutput=False)
    wp01 = nc.declare_dram_parameter("wp01", [128, C], BF16, isOutput=False)
    wp2 = nc.declare_dram_parameter("wp2", [64, C], BF16, isOutput=False)
    # consts: [onesp(2) | sel4(512) | ident(128)]
    consts = nc.declare_dram_parameter("consts", [128, 642], BF16,
                                       isOutput=False)
    bqk = nc.declare_dram_parameter("bqk", [128, HP], F32, isOutput=False)
    out = nc.declare_dram_parameter("out", [N, C], BF16, isOutput=True)

    with tile.TileContext(nc) as tc, ExitStack() as ctx:
        sb = ctx.enter_context(tc.tile_pool(name="sb", bufs=1))
        pipe = ctx.enter_context(tc.tile_pool(name="pipe", bufs=2))
        pxp = pipe     # per-tag bufs below
        otp = pipe
        pop = pipe
        # PSUM: 4 + 2 + 1 + 1 = 8 banks, one pool with per-tag bufs
        sp = ctx.enter_context(tc.tile_pool(name="sp", bufs=2, space="PSUM"))
        qp = sp
        op = sp
        mp = sp

        # ---------- static SBUF tiles ----------
        xs = sb.tile([128, CCH, N], BF16, tag="xs")
        wqk_sb = sb.tile([128, CCH, HP * 128], BF16, tag="wqk")
        wv_sb = sb.tile([128, CCH, HP * 64], BF16, tag="wv")
        cos_sb = sb.tile([128, N], BF16, tag="cos")
        sin_sb = sb.tile([128, N], BF16, tag="sin")
        cn = sb.tile([128, 642], BF16, tag="cn")
        onesp_sb = cn[:, 0:2]
        sel_sb = cn[:, 2:514]
        ident_sb = cn[:, 514:642]
        bqk_sb = sb.tile([128, HP], F32, tag="bqk")
        wp01_sb = sb.tile([128, C], BF16, tag="wp01")
        wp2_sb = sb.tile([64, C], BF16, tag="wp2")

        q12 = sb.tile([128, N], BF16, tag="q12")
        k12 = sb.tile([128, N], BF16, tag="k12")
        q3 = sb.tile([64, N], BF16, tag="q3")
        k3 = sb.tile([64, N], BF16, tag="k3")
        t4_all = sb.tile([128, N], BF16, tag="t4_all")
        s_sb = sb.tile([128, 512], F32, tag="s_sb")
        sv = sb.tile([128, 512], BF16, tag="sv")
        v3 = sb.tile([128, KB, HP, 65], BF16, tag="v3")
        ones48 = sb.tile([128, KB * HP], BF16, tag="ones48")
        o2 = sb.tile([128, NT, 4, 128], BF16, tag="o2")
        o1 = sb.tile([128, NT, 4, 64], BF16, tag="o1")

        def qT(h):
            return (q12[0:64], q12[64:128], q3[:])[h]

        def kT(h):
            return (k12[0:64], k12[64:128], k3[:])[h]

        # ---------- prologue DMAs (ordered for earliest qkv start) ----------
        xT_r = xT[:].rearrange("(c p) n -> p c n", p=128)
        wqk_r = wqk[:].rearrange("(c p) m -> p c m", p=128)
        d = nc.sync.dma_start
        d(wqk_sb[:, 0:2, :], wqk_r[:, 0:2, :])
        d(xs[:, 0:3, 0:512], xT_r[:, 0:3, 0:512])      # tile-0 tokens
        d(wqk_sb[:, 2:6, :], wqk_r[:, 2:6, :])
        d(xs[:, 3:6, 0:512], xT_r[:, 3:6, 0:512])
        d(bqk_sb[:], bqk[:, :])
        d(cn[:], consts[:, :])
        d(cos_sb[:, 0:1024], cosw[:, 0:1024])
        d(sin_sb[:, 0:1024], sinw[:, 0:1024])
        d(xs[:, :, 512:1024], xT_r[:, :, 512:1024])
        d(xs[:, :, 1024:1536], xT_r[:, :, 1024:1536])
        d(wv_sb[:].rearrange("p c m -> p (c m)"), wvp[:, :])
        d(xs[:, :, 1536:2048], xT_r[:, :, 1536:2048])
        d(cos_sb[:, 1024:2048], cosw[:, 1024:2048])
        d(sin_sb[:, 1024:2048], sinw[:, 1024:2048])
        d(wp01_sb[:], wp01[:, :])
        d(wp2_sb[:], wp2[:, :])

        nc.vector.memset(sv[:], 1.0)   # rows never written stay 1 (sel zeros them)
        nc.vector.memset(s_sb[:], 1.0)
        nc.vector.memset(ones48[:], 1.0)
        nc.vector.tensor_copy(
            v3[:].rearrange("p a b n -> p (a b) n", n=65)[:, :, 64], ones48[:])

        def mm(out_ap, lhsT, rhs, start, stop):
            nc.tensor.matmul(out_ap, lhsT, rhs, start=start, stop=stop,
                             skip_group_check=True)

        # ---------- qkv-head generator ----------
        # Per-tile chunks: mms -> RoPE pipe -> finA (sumsq+rsqrt) ->
        # finB (broadcast+scale). Emission defers fins so PE stays dense;
        # the qp ring (2) tolerates exactly one deferred finA.
        def qkv_gen(h):
            hs = slice(h * 128, (h + 1) * 128)
            qk = [None] * NT

            def mms(t):
                ts = slice(t * 512, (t + 1) * 512)
                qk[t] = qp.tile([128, 512], F32, tag="q", name=f"qk{t}")
                for c in range(CCH):
                    mm(qk[t][:], wqk_sb[:, c, hs], xs[:, c, ts], c == 0,
                       c == CCH - 1)

            def rope(t):
                ts = slice(t * 512, (t + 1) * 512)
                qkb = pipe.tile([128, 512], BF16, tag="qkb")
                nc.vector.tensor_scalar(qkb[:], qk[t][:], bqk_sb[:, h:h + 1],
                                        None, ALU.add)
                sq = pipe.tile([128, 512], BF16, tag="sq")
                if h == 0:
                    nc.vector.tensor_mul(sq[:], qkb[:], qkb[:])
                t1 = pipe.tile([128, 512], BF16, tag="t1")
                nc.gpsimd.tensor_mul(t1[:], qkb[:], cos_sb[:, ts])
                t2 = pipe.tile([128, 512], BF16, tag="t2")
                nc.vector.stream_shuffle(t2[:], qkb[:], SWAP_MASK)
                t3 = pipe.tile([128, 512], BF16, tag="t3")
                nc.vector.tensor_mul(t3[:], t2[:], sin_sb[:, ts])
                nc.vector.tensor_add(t4_all[:, ts], t1[:], t3[:])
                if h != 0:
                    nc.gpsimd.tensor_mul(sq[:], qkb[:], qkb[:])
                return sq

            def finA(t, sq):
                rows = slice(32 * t, 32 * t + 2)
                mm(qk[t][0:2, :], onesp_sb[:], sq[:], True, True)
                if h == 0:
                    # rsqrt = exp(-0.5 ln(ms)); same ACT table as softmax exp
                    lv = pipe.tile([2, 512], F32, tag="lv", name="lv")
                    nc.scalar.activation(lv[:], qk[t][0:2, :], AF.Ln,
                                         bias=0.0, scale=1.0 / HD)
                    nc.scalar.activation(sv[rows, :], lv[:], AF.Exp,
                                         bias=0.0, scale=-0.5)
                else:
                    nc.vector.tensor_copy(s_sb[rows, :], qk[t][0:2, :])

            def lnexp():
                lva = pipe.tile([128, 512], F32, tag="lva", name="lva")
                nc.scalar.activation(lva[:], s_sb[:], AF.Ln,
                                     bias=0.0, scale=1.0 / HD)
                nc.scalar.activation(sv[:], lva[:], AF.Exp, bias=0.0,
                                     scale=-0.5)

            def finB(t):
                ts = slice(t * 512, (t + 1) * 512)
                sqk_ps = qp.tile([128, 512], F32, tag="q")
                mm(sqk_ps[:], sel_sb[:, t * 128:(t + 1) * 128], sv[:],
                   True, True)
                nc.vector.tensor_mul(qT(h)[:, ts], t4_all[0:64, ts],
                                     sqk_ps[0:64, :])
                nc.vector.tensor_mul(kT(h)[:, ts], t4_all[64:128, ts],
                                     sqk_ps[64:128, :])

            sqs = [None] * NT

            def do_mms(t):
                mms(t)
                sqs[t] = rope(t)

            do_mms(0)
            yield 4500
            do_mms(1)
            yield 4500
            finA(0, sqs[0])
            yield 700
            do_mms(2)
            yield 4500
            finA(1, sqs[1])
            if h == 0:
                finB(0)
            yield 1600
            finA(2, sqs[2])
            yield 700
            do_mms(3)
            yield 4500
            if h == 0:
                finB(1)
            yield 900
            finA(3, sqs[3])
            yield 700
            if h != 0:
                lnexp()
                yield 700
                finB(0)
                yield 900
                finB(1)
                yield 900
            finB(2)
            yield 900
            finB(3)
            yield 900

        # ---------- v generator ----------
        def v_gen():
            for tt in range(KB):
                v_ps = qp.tile([128, HP * 64], F32, tag="q")
                for c in range(CCH):
                    mm(v_ps[:], xs[:, c, tt * 128:(tt + 1) * 128],
                       wv_sb[:, c, :], c == 0, c == CCH - 1)
                nc.vector.tensor_copy(
                    v3[:, tt, :, 0:64],
                    v_ps[:, :].rearrange("p (h n) -> p h n", h=HP))
                yield 1500

        # ---------- proj of one (qtile, qblock) ----------
        mtr = [None]

        def proj_qb(qt, qb):
            if mtr[0] is None:
                mtr[0] = mp.tile([128, 4, 128], F32, tag="m", name="mtr", bufs=1)
            m = mtr[0]
            tr01 = m[:, qb, 0:64].bitcast(BF16)
            tr2 = m[0:64, qb, 64:128].bitcast(BF16)
            nc.tensor.transpose(tr01, o2[:, qt, qb, :], ident_sb[:])
            nc.tensor.transpose(tr2, o1[:, qt, qb, :], ident_sb[:])
            on_act = qt == 3   # ACT is idle once the last exps drain
            ot01 = otp.tile([128, 128], BF16, tag="ot01", bufs=6)
            ot2 = otp.tile([64, 128], BF16, tag="ot2", bufs=6)
            if on_act:
                nc.scalar.activation(ot01[:], tr01, AF.Copy, bias=0.0,
                                     scale=1.0)
                nc.vector.tensor_copy(ot2[:], tr2)
            else:
                nc.vector.tensor_copy(ot01[:], tr01)
                nc.vector.tensor_copy(ot2[:], tr2)
            po = pop.tile([128, C], BF16, tag="po", bufs=3)
            for half in range(2):
                cs = slice(half * 384, (half + 1) * 384)
                p_ps = qp.tile([128, 384], F32, tag="q")
                mm(p_ps[:], ot01[:], wp01_sb[:, cs], True, False)
                mm(p_ps[:], ot2[:], wp2_sb[:, cs], False, True)
                if on_act and half == 1:
                    nc.scalar.activation(po[:, cs], p_ps[:], AF.Copy,
                                         bias=0.0, scale=1.0)
                else:
                    nc.vector.tensor_copy(po[:, cs], p_ps[:])
            tb = qt * 4 + qb
            nc.sync.dma_start(out[tb * 128:(tb + 1) * 128, :], po[:])

        # ---------- filler pump ----------
        fillers = deque()
        debt = [0.0]

        def pump(budget):
            budget += debt[0]
            while budget > 0 and fillers:
                try:
                    budget -= next(fillers[0])
                except StopIteration:
                    fillers.popleft()
            debt[0] = min(budget, 3000.0)

        def ensure_done(gen):
            """Pump until `gen` has fully emitted (emission-order guard for
            cross-generator data deps)."""
            while gen in fillers:
                pump(100000)

        # ---------- attention stream ----------
        phases = [(h, qt) for h in range(HP) for qt in range(NT)]
        px_tiles = {}
        emitted = set()

        def emit_group(p, g):
            if (p, g) in emitted:
                return
            emitted.add((p, g))
            h, qt = phases[p]
            qs = slice(qt * 512, (qt + 1) * 512)
            s_ps = sp.tile([128, 1024], F32, tag="s")
            for j in range(2):
                kb = 2 * g + j
                mm(s_ps[:, j * 512:(j + 1) * 512],
                   kT(h)[:, kb * 128:(kb + 1) * 128], qT(h)[:, qs],
                   True, True)
            px = pxp.tile([128, 1024], BF16, tag="px", bufs=28)
            nc.scalar.activation(px[:], s_ps[:], AF.Exp, bias=0.0, scale=0.125)
            px_tiles[(p, g)] = px

        def close_gen(p):
            if p == 0:
                ensure_done(vg)   # PV reads v3; emission-order guard
            h, qt = phases[p]
            o_ps = op.tile([128, 4, 65], F32, tag="o", bufs=1)
            for qb in range(4):
                for g in range(NG):
                    px = px_tiles[(p, g)]
                    for j in range(2):
                        kb = 2 * g + j
                        mm(o_ps[:, qb, :],
                           px[:, j * 512 + qb * 128:j * 512 + (qb + 1) * 128],
                           v3[:, kb, h, :],
                           qb == 0 and kb == 0, kb == KB - 1)
                if qb == 1 or qb == 3:
                    yield
            # normalize by the ones-column denominators (batched reciprocal,
            # then per-qb per-partition multiply); epilogues after ALL PV so
            # coarse WAR tracking can't serialize the qb bundles
            rec4 = pipe.tile([128, 4], F32, tag="rec4", name="rec4")
            nc.vector.reciprocal(rec4[:], o_ps[:, :, 64])
            for qb in range(4):
                dst = (o2[:, qt, qb, h * 64:(h + 1) * 64] if h < 2
                       else o1[:, qt, qb, :])
                nc.vector.tensor_scalar(dst, o_ps[:, qb, 0:64],
                                        rec4[:, qb:qb + 1], None, ALU.mult)
                if h == 2:
                    proj_qb(qt, qb)
                yield
            for g in range(NG):
                del px_tiles[(p, g)]

        def drain_close(cg):
            for _ in cg:
                pass

        # ---------- main schedule ----------
        # Phase 0 runs with qkv(h0) inlined per tile: tile t unlocks S groups
        # 2t, 2t+1 (k-tiles) while qT(qt0) comes entirely from tile 0.
        qg0 = qkv_gen(0)
        vg = v_gen()
        g1, g2 = qkv_gen(1), qkv_gen(2)
        fillers.append(vg)
        fillers.append(g1)
        need_gen = {1: g1, 2: g2}

        def drain_n(gen, n_chunks):
            for _ in range(n_chunks):
                next(gen)

        # Front: dense qkv-h0 mms with the exp stream fed by EVERY group
        # whose gates are open. Tile t gates k-blocks 4t..4t+3 (groups
        # 2t,2t+1 of every h0 phase) and the q-tokens of phase (0,t).
        drain_n(qg0, 5)              # thru finB(0)
        front = [(0, 0), (0, 1),
                 "T1", (0, 2), (0, 3), (1, 0), (1, 1),
                 "T2", (0, 4), (0, 5), (1, 2), (1, 3), (2, 0), (2, 1),
                 "T3", (0, 6), (0, 7), (1, 4), (1, 5), (2, 2), (2, 3),
                 (3, 0), (3, 1)]
        for item in front:
            if item == "T1":
                drain_n(qg0, 3)      # finA2, mms3, finB1
            elif item == "T2":
                drain_n(qg0, 2)      # finA3, finB2
            elif item == "T3":
                drain_n(qg0, 1)      # finB3
            else:
                emit_group(*item)
                pump(600)

        # Steady state: early closes deferred ~2 phases (px ring holds ~3
        # phases) so v/qkv fillers use the early PE slack; late closes pulled
        # in so proj work overlaps the remaining exp stream.
        NP = len(phases)
        close_at = {}
        for p in range(NP - 1):
            # early closes deferred 3 phases (shifts PV out of the PE-heavy
            # qkv/v window); h2 closes pulled in so proj overlaps exps
            lag = (p + 3, 1) if p < 8 else (p + 1, 0)
            close_at.setdefault(lag, []).append(p)

        def after_close(cp):
            if phases[cp] == (0, 2):
                fillers.append(g2)

        active_closes = []
        for p in range(1, NP):
            nh = phases[p][0]
            if nh != phases[p - 1][0]:
                ensure_done(need_gen[nh])
            for g in range(NG):
                emit_group(p, g)
                pump(2400 if p <= 4 else 2200)
                for cp in close_at.get((p, g), []):
                    active_closes.append((cp, close_gen(cp)))
                if active_closes:
                    steps = 2 if p >= NP - 2 else 1
                    for _ in range(steps):
                        if not active_closes:
                            break
                        cp, cg = active_closes[0]
                        try:
                            next(cg)
                        except StopIteration:
                            active_closes.pop(0)
                            after_close(cp)
        for cp, cg in active_closes:
            drain_close(cg)
            after_close(cp)
        drain_close(close_gen(NP - 1))
        while fillers:
            pump(100000)

    if split_waits:
        _split_waits(nc)
    return nc


def _split_waits(nc):
    """Walrus lowers at most one sync-wait per instruction; move excess waits
    onto NoOps inserted just before, on the same engine queue."""
    k = 0
    for fn in nc.m.functions:
        for bb in fn.blocks:
            il = bb.instructions
            idx = 0
            while idx < len(il):
                inst = il[idx]
                si = inst.sync_info
                eng = getattr(inst, "engine", None)
                if (si is not None and len(si.on_wait) > 1
                        and eng is not None
                        and str(eng) != "EngineType.Unassigned"):
                    waits = list(si.on_wait)
                    inst.sync_info = mybir.SyncInfo(
                        on_wait=[waits[-1]], on_update=list(si.on_update))
                    for w in waits[:-1]:
                        nop = mybir.InstNoOp(
                            name=f"I-waitnop-{k}", engine=eng, ins=[], outs=[],
                            sync_info=mybir.SyncInfo(on_wait=[w], on_update=[]))
                        k += 1
                        il.insert(idx, nop)
                        idx += 1
                idx += 1


def _prep_core_inputs(core, x, rope_cos, rope_sin, qkv_kernel, qkv_bias,
                      proj_kernel, proj_bias, q_norm_w, k_norm_w):
    b = core // 4
    heads = [3 * (core % 4) + i for i in range(HP)]

    wq = qkv_kernel.reshape(C, 3, H, HD)
    bq = qkv_bias.reshape(3, H, HD)

    xTa = np.ascontiguousarray(x[b].T).astype(BF)

    wqk = np.empty((C, HP * 128), np.float32)
    bqk = np.zeros((128, HP), np.float32)
    for i, h in enumerate(heads):
        wqk[:, i * 128:i * 128 + 64] = wq[:, 0, h, PERM]
        wqk[:, i * 128 + 64:(i + 1) * 128] = wq[:, 1, h, PERM]
        bqk[0:64, i] = bq[0, h, PERM]
        bqk[64:128, i] = bq[1, h, PERM]

    wv = np.zeros((C, HP * 64), np.float32)
    for i, h in enumerate(heads):
        wv[:, i * 64:(i + 1) * 64] = wq[:, 2, h, :]
    # packed [p, (c m)] so the SBUF copy is one dense DMA
    wvp = wv.reshape(CCH, 128, HP * 64).transpose(1, 0, 2).reshape(128, -1)

    cosT = rope_cos.T  # (HD, N)
    sinT = rope_sin.T
    cosw = np.empty((128, N), np.float32)
    sinw = np.empty((128, N), np.float32)
    cosw[0:64] = cosT[PERM] * q_norm_w[PERM][:, None]
    cosw[64:128] = cosT[PERM] * k_norm_w[PERM][:, None]
    # sin multiplies the SHUFFLED (partner) value -> partner's norm weight
    qn_p = q_norm_w[PERM][SWAPIDX]
    kn_p = k_norm_w[PERM][SWAPIDX]
    sinw[0:64] = SIGN[:, None] * sinT[PERM] * qn_p[:, None]
    sinw[64:128] = SIGN[:, None] * sinT[PERM] * kn_p[:, None]

    onesp = np.zeros((128, 2), np.float32)
    onesp[0:64, 0] = 1.0
    onesp[64:128, 1] = 1.0

    sel4 = np.zeros((128, 512), np.float32)
    for t in range(NT):
        sel4[32 * t, t * 128:t * 128 + 64] = 1.0
        sel4[32 * t + 1, t * 128 + 64:(t + 1) * 128] = 1.0

    rows01 = np.concatenate([np.arange(h * HD, (h + 1) * HD)
                             for h in heads[0:2]])
    rows2 = np.arange(heads[2] * HD, (heads[2] + 1) * HD)
    wp01 = proj_kernel[rows01, :]
    wp2 = proj_kernel[rows2, :]

    consts = np.zeros((128, 642), np.float32)
    consts[:, 0:2] = onesp
    consts[:, 2:514] = sel4
    consts[:, 514:642] = np.eye(128, dtype=np.float32)
    return {"xT": xTa, "wqk": wqk.astype(BF), "bqk": bqk,
            "cosw": cosw.astype(BF), "sinw": sinw.astype(BF),
            "wvp": np.ascontiguousarray(wvp).astype(BF),
            "wp01": np.ascontiguousarray(wp01).astype(BF),
            "wp2": np.ascontiguousarray(wp2).astype(BF),
            "consts": consts.astype(BF)}


def kernel(x, rope_cos, rope_sin, qkv_kernel, qkv_bias, proj_kernel,
           proj_bias, q_norm_w, k_norm_w, _trace=False):
    args = [np.asarray(a, dtype=np.float32) for a in
            (x, rope_cos, rope_sin, qkv_kernel, qkv_bias, proj_kernel,
             proj_bias, q_norm_w, k_norm_w)]
    in_maps = [_prep_core_inputs(c, *args) for c in range(NCORES)]

    if "nc" not in _NC_CACHE:
        _NC_CACHE["nc"] = build_nc()
    nc = _NC_CACHE["nc"]

    res = run_bass_kernel_spmd(nc, in_maps, core_ids=list(range(NCORES)),
                               trace=_trace)
    parts = [np.asarray(res.results[c]["out"]).astype(np.float32)
             for c in range(NCORES)]
    # v-bias contributes exactly bv @ proj_kernel (softmax rows sum to 1)
    pb = (np.asarray(proj_bias, dtype=np.float32)
          + np.asarray(qkv_bias, dtype=np.float32)[2 * C:]
          @ np.asarray(proj_kernel, dtype=np.float32))
    out = np.empty((B, N, C), np.float32)
    for b in range(B):
        out[b] = (parts[4 * b] + parts[4 * b + 1] + parts[4 * b + 2]
                  + parts[4 * b + 3] + pb)
    if _trace:
        kernel.last_results = res
    return out
